# revision 1
# baseline (speedup 1.0000x reference)
"""DiT block kernel for Trainium2, data-parallel over batch (8 cores, B=8).

v2: fp8 DoubleRow matmuls for qkv/scores/AV/proj/mlp1/mlp2 (2 K-chunks per
pass), LN statistics and adaLN GEMV computed with x/w_ada as the *stationary*
operand and a tiny moving operand (cost ~ output free size), per-token
quantities (LN rstd, softmax 1/den) produced directly in token-on-partition
layout via PE transposes of single rows, elementwise work spread across
DVE / ACT / GPSIMD.

Layouts (host-retiled):
  xT8      [128, 8, 1024]  bf16  x[b].T tiled: [p, kc, s] = x[b, s, kc*128+p]
  w_qkv8   [128, 4, 2, 3072] fp8 (x32), k/q column blocks permuted so that
           head h lives on partitions 32*(h%4).. with c split across the
           DoubleRow slot dim (c%32 on partitions, c//32 on slot)
  w_proj8  [128, 4, 2, 1024] fp8 (x32), rows permuted to match the y8 layout
           head h -> tile h//4, slot (h%4)//2, partition base 64*(h%2)
  w_mlp18  [128, 4, 2, 4096] fp8 (x32)
  w_mlp28  [128, 16, 2, 1024] fp8 (x32)
  w_ada_t  [128, 6, 8, 1024] bf16  [p, blk, kc, m] = w_ada[kc*128+p, blk*1024+m]
  outT8    [128, 8, 1024]  f32   [p, mc, s] = out[b, s, mc*128+p]

Scale bookkeeping: weights x32 in fp8. scoresT psum = 1024*k.q -> exp scale
1/(64*1024). AV y8 = 16*y_true (rd = 0.5/den). proj psum = 512*attn -> gate/512.
mlp1 psum = 32*pre -> gelu scale 1/32. mlp2 psum = 32*mlp -> gate/32.
"""

import os
import sys
import functools
from contextlib import ExitStack

import numpy as np

for _p in ("/opt/trn_rl_repo", "/root/.axon_site/_ro/trn_rl_repo"):
    if os.path.isdir(_p) and _p not in sys.path:
        sys.path.insert(0, _p)

import ml_dtypes  # noqa: E402
import concourse.bass as bass  # noqa: E402
from concourse import bacc  # noqa: E402
import concourse.tile as tile  # noqa: E402
from concourse import mybir  # noqa: E402
from concourse.bass_utils import run_bass_kernel_spmd  # noqa: E402

F32 = mybir.dt.float32
BF16 = mybir.dt.bfloat16
FP8 = mybir.dt.float8e4
AF = mybir.ActivationFunctionType
OP = mybir.AluOpType
DR = mybir.MatmulPerfMode.DoubleRow

B, S, H, NH, CH = 8, 1024, 1024, 16, 64
P = 128
KH = H // P          # 8 chunks over H
KS = S // P          # 8 chunks over S
EPS = 1e-6
N_CORES = 8
WS = 32.0            # fp8 weight scale

# scr2 (bf16) scratch layout: LN r/mr (2 LNs x 2 rows x S) then per-head rd
SCR2_LN = 0
SCR2_HEAD = 4 * S
SCR2_N = SCR2_HEAD + NH * S


def _build_program():
    nc = bacc.Bacc("TRN2", target_bir_lowering=False, debug=False)

    t = {}
    t["xT8"] = nc.dram_tensor("xT8", (P, KH, S), BF16, kind="ExternalInput").ap()
    t["cvec"] = nc.dram_tensor("cvec", (H,), F32, kind="ExternalInput").ap()
    t["w_ada_t"] = nc.dram_tensor("w_ada_t", (P, 6, KH, 1024), BF16,
                                  kind="ExternalInput").ap()
    t["b_ada"] = nc.dram_tensor("b_ada", (6 * H,), F32, kind="ExternalInput").ap()
    t["w_qkv8"] = nc.dram_tensor("w_qkv8", (P, 4, 2, 3 * H), FP8,
                                 kind="ExternalInput").ap()
    t["w_proj8"] = nc.dram_tensor("w_proj8", (P, 4, 2, H), FP8,
                                  kind="ExternalInput").ap()
    t["b_proj"] = nc.dram_tensor("b_proj", (H,), F32, kind="ExternalInput").ap()
    t["w_mlp18"] = nc.dram_tensor("w_mlp18", (P, 4, 2, 4 * H), FP8,
                                  kind="ExternalInput").ap()
    t["w_mlp18l"] = nc.dram_tensor("w_mlp18l", (P, 4, 2, 4 * H), FP8,
                                   kind="ExternalInput").ap()
    t["b_mlp1"] = nc.dram_tensor("b_mlp1", (4 * H,), F32, kind="ExternalInput").ap()
    t["w_mlp28"] = nc.dram_tensor("w_mlp28", (P, 16, 2, H), FP8,
                                  kind="ExternalInput").ap()
    t["w_mlp28l"] = nc.dram_tensor("w_mlp28l", (P, 16, 2, H), FP8,
                                   kind="ExternalInput").ap()
    t["b_mlp2"] = nc.dram_tensor("b_mlp2", (H,), F32, kind="ExternalInput").ap()
    t["outT8"] = nc.dram_tensor("outT8", (P, KH, S), F32, kind="ExternalOutput").ap()
    t["scr2"] = nc.dram_tensor("scr2", (SCR2_N,), BF16, kind="ExternalOutput").ap()

    with tile.TileContext(nc) as tc:
        _emit(tc, t)
    nc.compile()
    return nc


def _emit(tc, t):
    nc = tc.nc
    scr2 = t["scr2"]

    def pbcast(ap_1p, nparts):
        """Partition-broadcast view of a 1-partition (DRAM) AP."""
        return bass.AP(
            tensor=ap_1p.tensor, offset=ap_1p.offset,
            ap=[[0, nparts]] + list(ap_1p.ap[1:]),
        )

    def scr2_row(off, n):
        return scr2[off:off + n].rearrange("(a n) -> a n", a=1)

    def scr2_tok(off, n):
        """[128, n//128] view; flat[k*128+p] = element [p, k]."""
        return scr2[off:off + n].rearrange("(k p) -> p k", p=P)

    with ExitStack() as ctx:
        const = ctx.enter_context(tc.tile_pool(name="const", bufs=1))
        rows = ctx.enter_context(tc.tile_pool(name="rows", bufs=1))
        work = ctx.enter_context(tc.tile_pool(name="work", bufs=2))
        xpool = ctx.enter_context(tc.tile_pool(name="xpool", bufs=1))
        bcast = ctx.enter_context(tc.tile_pool(name="bcast", bufs=1))
        wmlp1_pool = ctx.enter_context(tc.tile_pool(name="wmlp1", bufs=1))
        wmlp1 = wmlp1_pool.tile([P, 4, 2, 4 * H], FP8, tag="wmlp1")
        wmlp1l = wmlp1_pool.tile([P, 4, 2, 4 * H], FP8, tag="wmlp1l")
        wada_ctx = ExitStack()
        wada_pool = wada_ctx.enter_context(tc.tile_pool(name="wada", bufs=1))

        # ---------------- constants ----------------------------------------
        ones_mv = const.tile([P, 1], BF16, tag="ones_mv")
        nc.vector.memset(ones_mv, 1.0)

        c_sb = const.tile([P, KH], F32, tag="c_sb")
        nc.gpsimd.dma_start(c_sb, t["cvec"].rearrange("(k p) -> p k", p=P))
        b_ada_sb = const.tile([P, 48], F32, tag="b_ada_sb")
        nc.gpsimd.dma_start(b_ada_sb, t["b_ada"].rearrange("(k p) -> p k", p=P))
        b_proj_sb = const.tile([P, KH], F32, tag="b_proj_sb")
        nc.gpsimd.dma_start(b_proj_sb, t["b_proj"].rearrange("(k p) -> p k", p=P))
        b_mlp1_sb = const.tile([P, 32], F32, tag="b_mlp1_sb")
        nc.gpsimd.dma_start(b_mlp1_sb, t["b_mlp1"].rearrange("(k p) -> p k", p=P))
        b_mlp2_sb = const.tile([P, KH], F32, tag="b_mlp2_sb")
        nc.gpsimd.dma_start(b_mlp2_sb, t["b_mlp2"].rearrange("(k p) -> p k", p=P))

        # ---------------- input x + first w_ada half-blocks ----------------
        xall = xpool.tile([P, KH, S], BF16, tag="xall")
        nc.sync.dma_start(xall[:, 0:4, :], t["xT8"][:, 0:4, :])
        nc.sync.dma_start(xall[:, 4:8, :], t["xT8"][:, 4:8, :])

        # w_ada streamed as 12 half-blocks [P, KH, 512] (4 cmod cols each)
        def wada_dma(hb, eng=None):
            wt = wada_pool.tile([P, KH, 512], BF16, tag="wada")
            blk, mlo = hb // 2, (hb % 2) * 512
            (eng or nc.gpsimd).dma_start(
                wt, t["w_ada_t"][:, blk, :, mlo:mlo + 512])
            return wt

        wada_tiles = {hb: wada_dma(hb, nc.sync) for hb in range(2)}

        # ---------------- silu(c) ------------------------------------------
        sc_sb = const.tile([P, KH], BF16, tag="sc_sb")
        nc.scalar.activation(sc_sb, c_sb, AF.Silu)

        # attention SBUF tiles (allocated early for pool stack order)
        att_ctx = ExitStack()
        kq_pool = att_ctx.enter_context(tc.tile_pool(name="kq", bufs=1))
        k8 = [kq_pool.tile([P, 2, S], FP8, tag=f"k8_{i}", name=f"k8_{i}")
              for i in range(KS)]
        q8 = [kq_pool.tile([P, 2, S], FP8, tag=f"q8_{i}", name=f"q8_{i}")
              for i in range(KS)]
        for i in range(KS):
            nc.vector.memset(k8[i][:, 1, :].bitcast(mybir.dt.uint32), 0)
            nc.vector.memset(q8[i][:, 1, :].bitcast(mybir.dt.uint32), 0)
        v2 = [kq_pool.tile([P, 2, NH // 2, 2, CH + 32], FP8, tag=f"v2_{i}",
                           name=f"v2_{i}")
              for i in range(4)]
        for i in range(4):
            nc.vector.memset(v2[i][:, :, :, :, CH:CH + 32], 2.0)
        y8 = [kq_pool.tile([P, 2, S], FP8, tag=f"y8_{i}", name=f"y8_{i}")
              for i in range(4)]

        # ---------------- phase-1 psum pools --------------------------------
        ph1 = ExitStack()
        ps_ada = ph1.enter_context(tc.tile_pool(name="ps_ada", bufs=1, space="PSUM"))
        psada = ps_ada.tile([P, 48], F32, tag="ada")
        cmod = const.tile([P, 48], F32, tag="cmod")

        ln1_ps = ExitStack()
        ps_ln = ln1_ps.enter_context(tc.tile_pool(name="ps_ln", bufs=1, space="PSUM"))

        # ---------------- LN statistics (x stationary, ones moving) --------
        def ln_stats(pool, src, xsq_tag):
            """Returns psum [128, KS*KH] partial sums & sumsq (col tcv*KH+kc),
            token s = tc*128 + p. Each matmul is an independent start/stop
            group: interleaved accumulation in one PSUM bank is NOT safe (the
            start flag marks the whole 2 KiB bank pending-zero, wiping other
            columns' later accumulating writes), but completed columns' data
            survives subsequent starts."""
            pss = pool.tile([P, KS * KH], F32, tag="ln_s")
            psq = pool.tile([P, KS * KH], F32, tag="ln_q")
            for kc in range(KH):
                xsq = work.tile([P, S], BF16, tag=xsq_tag)
                nc.scalar.activation(xsq, src[:, kc, :], AF.Square)
                for tcv in range(KS):
                    sl = slice(tcv * P, (tcv + 1) * P)
                    col = tcv * KH + kc
                    nc.tensor.matmul(
                        pss[:, col:col + 1], lhsT=src[:, kc, sl], rhs=ones_mv,
                        start=True, stop=True,
                    )
                    nc.tensor.matmul(
                        psq[:, col:col + 1], lhsT=xsq[:, sl], rhs=ones_mv,
                        start=True, stop=True,
                    )
            return pss, psq

        def ln_finish(pss, psq, o2_base):
            """rstd & mean*rstd from [128, KS] stats; bf16 via scr2 to
            partition-broadcast tiles [128, S]."""
            pssum = rows.tile([P, KS], F32, tag="pssum")
            nc.vector.tensor_reduce(
                pssum, pss.rearrange("p (t k) -> p t k", t=KS),
                axis=mybir.AxisListType.X, op=OP.add)
            psqs = rows.tile([P, KS], F32, tag="psqs")
            nc.vector.tensor_reduce(
                psqs, psq.rearrange("p (t k) -> p t k", t=KS),
                axis=mybir.AxisListType.X, op=OP.add)
            m = rows.tile([P, KS], F32, tag="m_tok")
            nc.vector.tensor_scalar(out=m, in0=pssum, scalar1=1.0 / H,
                                    scalar2=0.0, op0=OP.mult, op1=OP.bypass)
            v = rows.tile([P, KS], F32, tag="v_tok")
            nc.vector.tensor_scalar(out=v, in0=psqs, scalar1=1.0 / H,
                                    scalar2=EPS, op0=OP.mult, op1=OP.add)
            msq = rows.tile([P, KS], F32, tag="msq_tok")
            nc.vector.tensor_tensor(msq, m, m, OP.mult)
            nc.vector.tensor_tensor(v, v, msq, OP.subtract)
            r = rows.tile([P, KS], F32, tag="r_tok")
            nc.vector.tensor_scalar(out=r, in0=v, scalar1=-0.5, scalar2=1.5,
                                    op0=OP.mult, op1=OP.add)
            s = rows.tile([P, KS], F32, tag="s_tok")
            for _ in range(2):
                nc.vector.tensor_tensor(s, r, r, OP.mult)
                nc.vector.tensor_tensor(s, s, v, OP.mult)
                nc.vector.tensor_scalar(out=s, in0=s, scalar1=-0.5, scalar2=1.5,
                                        op0=OP.mult, op1=OP.add)
                nc.vector.tensor_tensor(r, r, s, OP.mult)
            nc.vector.tensor_tensor(m, m, r, OP.mult)  # m <- m * r
            rb16 = rows.tile([P, KS], BF16, tag="rb16")
            nc.vector.tensor_copy(rb16, r)
            mb16 = rows.tile([P, KS], BF16, tag="mb16")
            nc.vector.tensor_copy(mb16, m)
            nc.gpsimd.dma_start(scr2_tok(o2_base, S), rb16)
            nc.gpsimd.dma_start(scr2_tok(o2_base + S, S), mb16)
            r_b = bcast.tile([P, S], BF16, tag="r_b")
            mr_b = bcast.tile([P, S], BF16, tag="mr_b")
            nc.gpsimd.dma_start(r_b, pbcast(scr2_row(o2_base, S), P))
            nc.gpsimd.dma_start(mr_b, pbcast(scr2_row(o2_base + S, S), P))
            return r_b, mr_b

        pss1, psq1 = ln_stats(ps_ln, xall, "xsq")

        # adaLN GEMV: one half-block = 4 cmod columns of 128
        def ada_halfblock(hb, wt, ps, base):
            for mcol in range(4):
                col = hb * 4 + mcol - base
                for kc in range(KH):
                    nc.tensor.matmul(
                        ps[:, col:col + 1],
                        lhsT=wt[:, kc, mcol * P:(mcol + 1) * P],
                        rhs=sc_sb[:, kc:kc + 1],
                        start=(kc == 0), stop=(kc == KH - 1),
                    )
            nc.vector.tensor_tensor(
                cmod[:, hb * 4:(hb + 1) * 4],
                ps[:, hb * 4 - base:(hb + 1) * 4 - base],
                b_ada_sb[:, hb * 4:(hb + 1) * 4], OP.add,
            )

        for hb in range(2):  # shift_msa
            ada_halfblock(hb, wada_tiles.pop(hb), psada, 0)

        r1_b, mr1_b = ln_finish(pss1, psq1, SCR2_LN)
        for hb in range(2, 4):  # scale_msa, behind the r/mr loads in the FIFO
            ada_halfblock(hb, wada_dma(hb), psada, 0)
        ln1_ps.close()

        sc1 = const.tile([P, 16], F32, tag="sc1")  # 1+scale_msa | 1+scale_mlp
        nc.scalar.add(sc1[:, 0:8], cmod[:, 8:16], 1.0)

        # ---------------- z1 modulate + qkv (DoubleRow fp8) ----------------
        zpool = ExitStack()
        z1_pool = zpool.enter_context(tc.tile_pool(name="z1", bufs=1))
        z1 = [z1_pool.tile([P, 2, S], FP8, tag=f"z1_{i}", name=f"z1_{i}")
              for i in range(4)]

        wqkv_ctx = ExitStack()
        wqkv_pool = wqkv_ctx.enter_context(tc.tile_pool(name="wqkv", bufs=1))
        wqkv = wqkv_pool.tile([P, 4, 2, 3 * H], FP8, tag="wqkv")
        for sec in range(3):
            nc.gpsimd.dma_start(
                wqkv[:, :, :, sec * H:(sec + 1) * H],
                t["w_qkv8"][:, :, :, sec * H:(sec + 1) * H],
            )

        def modulate(dst, src, r_b, mr_b, col, shift_ap, sl):
            tm = work.tile([P, S], BF16, tag="mod_tm")
            nc.vector.tensor_tensor(tm[:, sl], src[:, sl], r_b[:, sl], OP.mult)
            nc.vector.tensor_tensor(tm[:, sl], tm[:, sl], mr_b[:, sl],
                                    OP.subtract)
            nc.scalar.activation(dst[:, sl], tm[:, sl], AF.Identity,
                                 bias=shift_ap, scale=sc1[:, col:col + 1])

        for half in range(2):
            sl = slice(half * 512, (half + 1) * 512)
            for kc in range(KH):
                modulate(z1[kc // 2][:, kc % 2, :], xall[:, kc, :], r1_b, mr1_b,
                         kc, cmod[:, kc:kc + 1], sl)

        ps_mm_ctx = ExitStack()
        ps_mm = ps_mm_ctx.enter_context(
            tc.tile_pool(name="ps_mm", bufs=3, space="PSUM"))

        for oc in range(16):  # 8 k-chunks then 8 q-chunks
            ps = ps_mm.tile([P, S], F32, tag="mm")
            for half in range(2):
                sl = slice(half * 512, (half + 1) * 512)
                for kcp in range(4):
                    nc.tensor.matmul(
                        ps[:, sl],
                        lhsT=wqkv[:, kcp, :, oc * P:(oc + 1) * P],
                        rhs=z1[kcp][:, :, sl],
                        start=(kcp == 0), stop=(kcp == 3), perf_mode=DR,
                    )
            dst = k8 if oc < 8 else q8
            nc.scalar.copy(dst[oc % 8][:, 0, :], ps)

        for sc in range(KS):  # v, token-major
            ps = ps_mm.tile([P, S], F32, tag="mm")
            for half in range(2):
                sl = slice(2048 + half * 512, 2048 + (half + 1) * 512)
                osl = slice(half * 512, (half + 1) * 512)
                for kcp in range(4):
                    nc.tensor.matmul(
                        ps[:, osl],
                        lhsT=z1[kcp][:, :, sc * P:(sc + 1) * P],
                        rhs=wqkv[:, kcp, :, sl],
                        start=(kcp == 0), stop=(kcp == 3), perf_mode=DR,
                    )
            nc.scalar.copy(
                v2[sc // 2][:, sc % 2, :, :, 0:CH],
                ps.rearrange("p (hp two c) -> p hp two c", hp=NH // 2, two=2),
            )
        ps_mm_ctx.close()
        wqkv_ctx.close()
        zpool.close()
        ph1.close()

        # ---------------- attention ----------------------------------------
        wexp_pool = att_ctx.enter_context(tc.tile_pool(name="wexp", bufs=8))
        att_tmp = att_ctx.enter_context(tc.tile_pool(name="att_tmp", bufs=3))
        rdb_pool = att_ctx.enter_context(tc.tile_pool(name="rdb", bufs=3))
        wproj_pool = att_ctx.enter_context(tc.tile_pool(name="wproj", bufs=1))
        wproj = wproj_pool.tile([P, 4, 2, H], FP8, tag="wproj")

        att_ps = ExitStack()
        spool = att_ps.enter_context(tc.tile_pool(name="spool", bufs=2, space="PSUM"))
        avpool = att_ps.enter_context(tc.tile_pool(name="avpool", bufs=2, space="PSUM"))

        DVE_EXP_KC = tuple(
            int(x) for x in os.environ.get("KEXPKC", "").split(",")
            if x != "")

        def head_scores(h):
            ti, off = h // 2, 64 * (h % 2)
            prow = slice(off, off + CH)
            wexp = []
            for kcp in range(4):
                wt = wexp_pool.tile([P, 2, S], FP8, tag="wexp")
                for j in range(2):
                    kc = 2 * kcp + j
                    ps_s = spool.tile([P, S], F32, tag="ps")
                    for half in range(2):
                        sl = slice(half * 512, (half + 1) * 512)
                        nc.tensor.matmul(
                            ps_s[:, sl],
                            lhsT=k8[ti][prow, :, kc * P:(kc + 1) * P],
                            rhs=q8[ti][prow, :, sl],
                            start=True, stop=True, perf_mode=DR,
                        )
                    if kc in DVE_EXP_KC:
                        # exp(t) ~ 1 + t(1 + t/2), |t| < 0.5 (err < 1e-3)
                        tq = work.tile([P, S], BF16, tag="mod_tm")
                        nc.vector.tensor_scalar(
                            out=tq, in0=ps_s,
                            scalar1=1.0 / (64.0 * WS * WS), scalar2=0.0,
                            op0=OP.mult, op1=OP.bypass)
                        uq = work.tile([P, S], BF16, tag="res_tmp")
                        nc.vector.tensor_scalar(
                            out=uq, in0=tq, scalar1=0.5, scalar2=1.0,
                            op0=OP.mult, op1=OP.add)
                        nc.vector.tensor_tensor(uq, tq, uq, OP.mult)
                        nc.vector.tensor_scalar(
                            out=wt[:, j, :], in0=uq, scalar1=1.0, scalar2=1.0,
                            op0=OP.mult, op1=OP.add)
                    else:
                        nc.scalar.activation(wt[:, j, :], ps_s, AF.Exp,
                                             scale=1.0 / (64.0 * WS * WS))
                wexp.append(wt)
            return wexp

        def head_av(h, wexp):
            """AV matmul with the softmax denominator fused in: the
            stationary operand is [v_head | 32 ones-cols valued 2.0], so
            output rows 0:64 are y_unnorm and rows 64:96 are 2*den — one
            DoubleRow group at base partition 0 (ISA-safe), no extra cost
            (matmul cost is output free size only)."""
            ps_y = avpool.tile([P, S], F32, tag="ps_y")
            for half in range(2):
                sl = slice(half * 512, (half + 1) * 512)
                for kcp in range(4):
                    nc.tensor.matmul(
                        ps_y[0:CH + 32, sl],
                        lhsT=v2[kcp][:, :, h // 2, h % 2, :],
                        rhs=wexp[kcp][:, :, sl],
                        start=(kcp == 0), stop=(kcp == 3), perf_mode=DR,
                    )
            drow = att_tmp.tile([1, S], BF16, tag="drow", bufs=2)
            with nc.allow_low_precision(reason="softmax 1/den in bf16"):
                nc.vector.reciprocal(drow, ps_y[CH:CH + 1, :])  # 0.5/den
            o2 = SCR2_HEAD + h * S
            nc.gpsimd.dma_start(scr2_row(o2, S), drow)
            rdb = rdb_pool.tile([P, S], BF16, tag="rdb")
            nc.gpsimd.dma_start(rdb, pbcast(scr2_row(o2, S), P))
            return ps_y, rdb

        def head_norm(h, ps_y, rdb):
            ti, j, off = h // 4, (h % 4) // 2, 64 * (h % 2)
            nc.vector.tensor_tensor(
                y8[ti][off:off + CH, j, :],
                ps_y[0:CH, :], rdb[0:CH, :], OP.mult,
            )

        def late_streams(step):
            if step == 2:
                nc.gpsimd.dma_start(wproj, t["w_proj8"])
            elif 4 <= step < 12:  # w_ada blocks 4..11
                wada_tiles[step] = wada_dma(step)
            elif 12 <= step < 20:  # w_mlp1 hi, 1MB pieces
                i = step - 12
                nc.gpsimd.dma_start(wmlp1[:, i // 2, :, (i % 2) * 2048:
                                          (i % 2) * 2048 + 2048],
                                    t["w_mlp18"][:, i // 2, :, (i % 2) * 2048:
                                                 (i % 2) * 2048 + 2048])
            elif 20 <= step < 28:  # w_mlp1 lo
                i = step - 20
                nc.gpsimd.dma_start(wmlp1l[:, i // 2, :, (i % 2) * 2048:
                                           (i % 2) * 2048 + 2048],
                                    t["w_mlp18l"][:, i // 2, :, (i % 2) * 2048:
                                                  (i % 2) * 2048 + 2048])

            if 6 <= step < 14:  # adaLN tail rides the scores psum ring
                hb = step - 2
                psx = spool.tile([P, S], F32, tag="ps")
                ada_halfblock(hb, wada_tiles.pop(hb), psx, hb * 4)

        st = {}
        for step in range(28):
            late_streams(step)
            if step < NH:
                st[step] = {"wexp": head_scores(step)}
            if 1 <= step and step - 1 < NH:
                hh = step - 1
                ps_y, rdb = head_av(hh, st[hh].pop("wexp"))
                st[hh]["ps_y"], st[hh]["rdb"] = ps_y, rdb
            if 2 <= step and step - 2 < NH:
                hh = step - 2
                head_norm(hh, st[hh].pop("ps_y"), st[hh].pop("rdb"))
                del st[hh]
        att_ps.close()


        nc.scalar.add(sc1[:, 8:16], cmod[:, 32:40], 1.0)
        gpr = const.tile([P, KH], F32, tag="gpr")
        nc.vector.tensor_scalar(out=gpr, in0=cmod[:, 16:24],
                                scalar1=1.0 / 512.0, scalar2=0.0,
                                op0=OP.mult, op1=OP.bypass)
        gpb = const.tile([P, KH], F32, tag="gpb")
        nc.vector.tensor_tensor(gpb, cmod[:, 16:24], b_proj_sb, OP.mult)
        gmr = const.tile([P, KH], F32, tag="gmr")
        nc.vector.tensor_scalar(out=gmr, in0=cmod[:, 40:48],
                                scalar1=1.0 / 32.0, scalar2=0.0,
                                op0=OP.mult, op1=OP.bypass)
        gmb = const.tile([P, KH], F32, tag="gmb")
        nc.vector.tensor_tensor(gmb, cmod[:, 40:48], b_mlp2_sb, OP.mult)

        # ---------------- proj + gated residual + LN2 stats -----------------
        ph3 = ExitStack()
        ps_pr = ph3.enter_context(tc.tile_pool(name="ps_pr", bufs=2, space="PSUM"))
        ps_ln2 = ph3.enter_context(tc.tile_pool(name="ps_ln2", bufs=1, space="PSUM"))
        pss2 = ps_ln2.tile([P, KS * KH], F32, tag="ln_s")
        psq2 = ps_ln2.tile([P, KS * KH], F32, tag="ln_q")
        for mc in range(KH):
            ps = ps_pr.tile([P, S], F32, tag="mm")
            for half in range(2):
                sl = slice(half * 512, (half + 1) * 512)
                for ti in range(4):
                    nc.tensor.matmul(
                        ps[:, sl],
                        lhsT=wproj[:, ti, :, mc * P:(mc + 1) * P],
                        rhs=y8[ti][:, :, sl],
                        start=(ti == 0), stop=(ti == 3), perf_mode=DR,
                    )
            tp = work.tile([P, S], BF16, tag="res_tmp")
            nc.scalar.activation(tp, ps, AF.Identity,
                                 bias=gpb[:, mc:mc + 1],
                                 scale=gpr[:, mc:mc + 1])
            nc.vector.tensor_tensor(xall[:, mc, :], xall[:, mc, :], tp, OP.add)
            # LN2 statistics for this chunk right away
            xsq = work.tile([P, S], BF16, tag="xsq")
            nc.vector.tensor_tensor(xsq, xall[:, mc, :], xall[:, mc, :],
                                    OP.mult)
            for tcv in range(KS):
                sl = slice(tcv * P, (tcv + 1) * P)
                col = tcv * KH + mc
                nc.tensor.matmul(
                    pss2[:, col:col + 1], lhsT=xall[:, mc, sl], rhs=ones_mv,
                    start=True, stop=True,
                )
                nc.tensor.matmul(
                    psq2[:, col:col + 1], lhsT=xsq[:, sl], rhs=ones_mv,
                    start=True, stop=True,
                )
        att_ctx.close()

        # ---------------- LN2 finish + modulate z2 + MLP --------------------
        ph4 = ExitStack()
        r2_b, mr2_b = ln_finish(pss2, psq2, SCR2_LN + 2 * S)
        ph3.close()
        wada_ctx.close()

        h_pool = ph4.enter_context(tc.tile_pool(name="h8", bufs=1))
        h8 = [h_pool.tile([P, 2, S], FP8, tag=f"h8_{i}", name=f"h8_{i}")
              for i in range(16)]
        h8l = [h_pool.tile([P, 2, S], FP8, tag=f"h8l_{i}", name=f"h8l_{i}")
               for i in range(16)]
        z2_pool = ph4.enter_context(tc.tile_pool(name="z2", bufs=1))
        z2 = [z2_pool.tile([P, 2, S], FP8, tag=f"z2_{i}", name=f"z2_{i}")
              for i in range(4)]
        z2l = [z2_pool.tile([P, 2, S], FP8, tag=f"z2l_{i}", name=f"z2l_{i}")
               for i in range(4)]
        for half in range(2):
            sl = slice(half * 512, (half + 1) * 512)
            for kc in range(KH):
                # z_bf (bf16) -> z_hi (fp8) -> z_lo = fp8(z_bf - z_hi)
                tm = work.tile([P, S], BF16, tag="mod_tm")
                nc.vector.tensor_tensor(tm[:, sl], xall[:, kc, sl],
                                        r2_b[:, sl], OP.mult)
                nc.vector.tensor_tensor(tm[:, sl], tm[:, sl], mr2_b[:, sl],
                                        OP.subtract)
                zbf = work.tile([P, S], BF16, tag="stage_bf")
                nc.vector.tensor_scalar(
                    out=zbf[:, sl], in0=tm[:, sl],
                    scalar1=sc1[:, 8 + kc:8 + kc + 1],
                    scalar2=cmod[:, 24 + kc:24 + kc + 1],
                    op0=OP.mult, op1=OP.add,
                )
                zhi = z2[kc // 2][:, kc % 2, :]
                nc.scalar.copy(zhi[:, sl], zbf[:, sl])
                nc.vector.tensor_tensor(z2l[kc // 2][:, kc % 2, sl],
                                        zbf[:, sl], zhi[:, sl], OP.subtract)

        wmlp2_pool = ph4.enter_context(tc.tile_pool(name="wmlp2", bufs=2))
        otmp_pool = ph4.enter_context(tc.tile_pool(name="otmp", bufs=2))

        def w2_blk_dma(mc):  # 1 out-chunk of hi+lo
            bh = wmlp2_pool.tile([P, 16, 2, P], FP8, tag="w2hi")
            nc.sync.dma_start(bh, t["w_mlp28"][:, :, :, mc * P:(mc + 1) * P])
            bl = wmlp2_pool.tile([P, 16, 2, P], FP8, tag="w2lo")
            nc.sync.dma_start(bl, t["w_mlp28l"][:, :, :, mc * P:(mc + 1) * P])
            return bh, bl

        w2blk = {0: w2_blk_dma(0), 1: w2_blk_dma(1)}

        m1_ctx = ExitStack()
        ps_m1 = m1_ctx.enter_context(tc.tile_pool(name="ps_m1", bufs=3, space="PSUM"))

        for mc in range(32):
            ps = ps_m1.tile([P, S], F32, tag="mm")
            for half in range(2):
                sl = slice(half * 512, (half + 1) * 512)
                for p_ in range(3):
                    wsrc = wmlp1 if p_ != 1 else wmlp1l
                    zsrc = z2 if p_ != 2 else z2l
                    for kcp in range(4):
                        nc.tensor.matmul(
                            ps[:, sl],
                            lhsT=wsrc[:, kcp, :, mc * P:(mc + 1) * P],
                            rhs=zsrc[kcp][:, :, sl],
                            start=(p_ == 0 and kcp == 0),
                            stop=(p_ == 2 and kcp == 3), perf_mode=DR,
                        )
            hbf = work.tile([P, S], BF16, tag="stage_bf")
            nc.scalar.activation(
                hbf, ps, AF.Gelu_apprx_tanh,
                bias=b_mlp1_sb[:, mc:mc + 1], scale=1.0 / WS,
            )
            hhi = h8[mc // 2][:, mc % 2, :]
            nc.vector.tensor_copy(hhi, hbf)
            nc.vector.tensor_tensor(h8l[mc // 2][:, mc % 2, :], hbf, hhi,
                                    OP.subtract)
        m1_ctx.close()

        w2_ctx = ExitStack()
        ps_m2 = w2_ctx.enter_context(tc.tile_pool(name="ps_m2", bufs=3, space="PSUM"))
        for mc in range(KH):
            if mc + 2 < KH:
                w2blk[mc + 2] = w2_blk_dma(mc + 2)
            bh, bl = w2blk.pop(mc)
            ps = ps_m2.tile([P, S], F32, tag="mm")
            off = 0
            for half in range(2):
                sl = slice(half * 512, (half + 1) * 512)
                for p_ in range(3):
                    wsrc = bh if p_ != 1 else bl
                    hsrc = h8 if p_ != 2 else h8l
                    for kcp in range(16):
                        nc.tensor.matmul(
                            ps[:, sl],
                            lhsT=wsrc[:, kcp, :, off:off + P],
                            rhs=hsrc[kcp][:, :, sl],
                            start=(p_ == 0 and kcp == 0),
                            stop=(p_ == 2 and kcp == 15), perf_mode=DR,
                        )
            tp = work.tile([P, S], BF16, tag="res_tmp")
            nc.scalar.activation(tp, ps, AF.Identity,
                                 bias=gmb[:, mc:mc + 1],
                                 scale=gmr[:, mc:mc + 1])
            ot = otmp_pool.tile([P, S], F32, tag="ot")
            nc.vector.tensor_tensor(ot, xall[:, mc, :], tp, OP.add)
            nc.sync.dma_start(t["outT8"][:, mc, :], ot)
        w2_ctx.close()
        ph4.close()


@functools.lru_cache(maxsize=1)
def _get_nc():
    return _build_program()


def _fp8(a):
    return np.ascontiguousarray(
        np.clip(np.asarray(a, dtype=np.float32), -240.0, 240.0)
        .astype(ml_dtypes.float8_e4m3))


def kernel(x, c, w_ada, b_ada, w_qkv, w_proj, b_proj, w_mlp1, b_mlp1,
           w_mlp2, b_mlp2):
    nc = _get_nc()
    bf = ml_dtypes.bfloat16
    f32 = np.float32

    p = np.arange(128)
    w_qkv8 = _fp8((np.asarray(w_qkv, f32) * WS)
                  .reshape(4, 2, 128, 3 * H).transpose(2, 0, 1, 3))

    # --- w_proj row permutation matching the y8 layout ---
    phi = np.empty((128, 4, 2), np.int64)
    for ti in range(4):
        for j in range(2):
            phi[:, ti, j] = (4 * ti + 2 * j + p // 64) * CH + (p % 64)
    w_proj8 = _fp8((np.asarray(w_proj, f32) * WS)[phi])

    w1s = (np.asarray(w_mlp1, f32) * WS).reshape(4, 2, 128, 4 * H)\
        .transpose(2, 0, 1, 3)
    w_mlp18 = _fp8(w1s)
    w_mlp18l = _fp8(w1s - w_mlp18.astype(f32))
    w2s = (np.asarray(w_mlp2, f32) * WS).reshape(16, 2, 128, H)\
        .transpose(2, 0, 1, 3)
    w_mlp28 = _fp8(w2s)
    w_mlp28l = _fp8(w2s - w_mlp28.astype(f32))
    w_ada_t = np.ascontiguousarray(
        np.asarray(w_ada, f32).reshape(8, 128, 6, 1024)
        .transpose(1, 2, 0, 3).astype(bf))

    shared = {
        "w_ada_t": w_ada_t,
        "b_ada": np.ascontiguousarray(b_ada, dtype=f32),
        "w_qkv8": w_qkv8,
        "w_proj8": w_proj8,
        "b_proj": np.ascontiguousarray(b_proj, dtype=f32),
        "w_mlp18": w_mlp18,
        "w_mlp18l": w_mlp18l,
        "b_mlp1": np.ascontiguousarray(b_mlp1, dtype=f32),
        "w_mlp28": w_mlp28,
        "w_mlp28l": w_mlp28l,
        "b_mlp2": np.ascontiguousarray(b_mlp2, dtype=f32),
    }
    in_maps = []
    for bidx in range(N_CORES):
        m = dict(shared)
        m["xT8"] = np.ascontiguousarray(
            np.asarray(x[bidx], f32).T.reshape(8, 128, S)
            .transpose(1, 0, 2).astype(bf))
        m["cvec"] = np.ascontiguousarray(np.asarray(c[bidx], dtype=f32))
        in_maps.append(m)

    res = run_bass_kernel_spmd(
        nc, in_maps, core_ids=list(range(N_CORES)), trace=False
    )
    kernel.last_results = res

    out = np.empty((B, S, H), dtype=f32)
    for bidx in range(N_CORES):
        o = np.asarray(res.results[bidx]["outT8"])  # [128, 8, S]
        out[bidx] = o.transpose(1, 0, 2).reshape(H, S).T
    return out


if __name__ == "__main__":
    nc = _get_nc()
    print("program built ok")



# revision 35
# speedup vs baseline: 1.0674x; 1.0674x over previous
"""DiT block kernel for Trainium2, data-parallel over batch (8 cores, B=8).

v2: fp8 DoubleRow matmuls for qkv/scores/AV/proj/mlp1/mlp2 (2 K-chunks per
pass), LN statistics and adaLN GEMV computed with x/w_ada as the *stationary*
operand and a tiny moving operand (cost ~ output free size), per-token
quantities (LN rstd, softmax 1/den) produced directly in token-on-partition
layout via PE transposes of single rows, elementwise work spread across
DVE / ACT / GPSIMD.

Layouts (host-retiled):
  xT8      [128, 8, 1024]  bf16  x[b].T tiled: [p, kc, s] = x[b, s, kc*128+p]
  w_qkv8   [128, 4, 2, 3072] fp8 (x32), k/q column blocks permuted so that
           head h lives on partitions 32*(h%4).. with c split across the
           DoubleRow slot dim (c%32 on partitions, c//32 on slot)
  w_proj8  [128, 4, 2, 1024] fp8 (x32), rows permuted to match the y8 layout
           head h -> tile h//4, slot (h%4)//2, partition base 64*(h%2)
  w_mlp18  [128, 4, 2, 4096] fp8 (x32)
  w_mlp28  [128, 16, 2, 1024] fp8 (x32)
  w_ada_t  [128, 6, 8, 1024] bf16  [p, blk, kc, m] = w_ada[kc*128+p, blk*1024+m]
  outT8    [128, 8, 1024]  f32   [p, mc, s] = out[b, s, mc*128+p]

Scale bookkeeping: weights x32 in fp8. scoresT psum = 1024*k.q -> exp scale
1/(64*1024). AV y8 = 16*y_true (rd = 0.5/den). proj psum = 512*attn -> gate/512.
mlp1 psum = 32*pre -> gelu scale 1/32. mlp2 psum = 32*mlp -> gate/32.
"""

import os
import sys
import functools
from contextlib import ExitStack

import numpy as np

for _p in ("/opt/trn_rl_repo", "/root/.axon_site/_ro/trn_rl_repo"):
    if os.path.isdir(_p) and _p not in sys.path:
        sys.path.insert(0, _p)

import ml_dtypes  # noqa: E402
import concourse.bass as bass  # noqa: E402
from concourse import bacc  # noqa: E402
import concourse.tile as tile  # noqa: E402
from concourse import mybir  # noqa: E402
from concourse.bass_utils import run_bass_kernel_spmd  # noqa: E402

F32 = mybir.dt.float32
BF16 = mybir.dt.bfloat16
FP8 = mybir.dt.float8e4
AF = mybir.ActivationFunctionType
OP = mybir.AluOpType
DR = mybir.MatmulPerfMode.DoubleRow

B, S, H, NH, CH = 8, 1024, 1024, 16, 64
P = 128
KH = H // P          # 8 chunks over H
KS = S // P          # 8 chunks over S
EPS = 1e-6
N_CORES = 8
WS = 32.0            # fp8 weight scale

# scr2 (bf16) scratch layout: LN r/mr (2 LNs x 2 rows x S) then per-head rd
SCR2_LN = 0
SCR2_HEAD = 4 * S
SCR2_N = SCR2_HEAD + NH * S


def _build_program():
    nc = bacc.Bacc("TRN2", target_bir_lowering=False, debug=False)

    t = {}
    t["xT8"] = nc.dram_tensor("xT8", (P, KH, S), BF16, kind="ExternalInput").ap()
    t["cvec"] = nc.dram_tensor("cvec", (H,), F32, kind="ExternalInput").ap()
    t["w_ada_t"] = nc.dram_tensor("w_ada_t", (P, 6, KH, 1024), BF16,
                                  kind="ExternalInput").ap()
    t["b_ada"] = nc.dram_tensor("b_ada", (6 * H,), F32, kind="ExternalInput").ap()
    t["w_qkv8"] = nc.dram_tensor("w_qkv8", (P, 4, 2, 3 * H), FP8,
                                 kind="ExternalInput").ap()
    t["w_proj8"] = nc.dram_tensor("w_proj8", (P, 4, 2, H), FP8,
                                  kind="ExternalInput").ap()
    t["b_proj"] = nc.dram_tensor("b_proj", (H,), F32, kind="ExternalInput").ap()
    t["w_mlp18"] = nc.dram_tensor("w_mlp18", (P, 4, 2, 4 * H), FP8,
                                  kind="ExternalInput").ap()
    t["w_mlp18l"] = nc.dram_tensor("w_mlp18l", (P, 4, 2, 4 * H), FP8,
                                   kind="ExternalInput").ap()
    t["b_mlp1"] = nc.dram_tensor("b_mlp1", (4 * H,), F32, kind="ExternalInput").ap()
    t["w_mlp28"] = nc.dram_tensor("w_mlp28", (P, 16, 2, H), FP8,
                                  kind="ExternalInput").ap()
    t["w_mlp28l"] = nc.dram_tensor("w_mlp28l", (P, 16, 2, H), FP8,
                                   kind="ExternalInput").ap()
    t["b_mlp2"] = nc.dram_tensor("b_mlp2", (H,), F32, kind="ExternalInput").ap()
    t["outT8"] = nc.dram_tensor("outT8", (P, KH, S), F32, kind="ExternalOutput").ap()
    t["scr2"] = nc.dram_tensor("scr2", (SCR2_N,), BF16, kind="ExternalOutput").ap()

    with tile.TileContext(nc) as tc:
        _emit(tc, t)
    nc.compile()
    return nc


def _emit(tc, t):
    nc = tc.nc
    scr2 = t["scr2"]

    def pbcast(ap_1p, nparts):
        """Partition-broadcast view of a 1-partition (DRAM) AP."""
        return bass.AP(
            tensor=ap_1p.tensor, offset=ap_1p.offset,
            ap=[[0, nparts]] + list(ap_1p.ap[1:]),
        )

    def scr2_row(off, n):
        return scr2[off:off + n].rearrange("(a n) -> a n", a=1)

    def scr2_tok(off, n):
        """[128, n//128] view; flat[k*128+p] = element [p, k]."""
        return scr2[off:off + n].rearrange("(k p) -> p k", p=P)

    with ExitStack() as ctx:
        const = ctx.enter_context(tc.tile_pool(name="const", bufs=1))
        rows = ctx.enter_context(tc.tile_pool(name="rows", bufs=1))
        work = ctx.enter_context(tc.tile_pool(name="work", bufs=2))
        xpool = ctx.enter_context(tc.tile_pool(name="xpool", bufs=1))
        bcast = ctx.enter_context(tc.tile_pool(name="bcast", bufs=1))
        wmlp1_pool = ctx.enter_context(tc.tile_pool(name="wmlp1", bufs=1))
        wmlp1 = wmlp1_pool.tile([P, 4, 2, 4 * H], FP8, tag="wmlp1")
        wmlp1l = wmlp1_pool.tile([P, 4, 2, 4 * H], FP8, tag="wmlp1l")
        wada_ctx = ExitStack()
        wada_pool = wada_ctx.enter_context(tc.tile_pool(name="wada", bufs=1))

        # ---------------- constants ----------------------------------------
        ones_mv = const.tile([P, 1], BF16, tag="ones_mv")
        nc.vector.memset(ones_mv, 1.0)

        c_sb = const.tile([P, KH], F32, tag="c_sb")
        nc.gpsimd.dma_start(c_sb, t["cvec"].rearrange("(k p) -> p k", p=P))
        b_ada_sb = const.tile([P, 48], F32, tag="b_ada_sb")
        nc.gpsimd.dma_start(b_ada_sb, t["b_ada"].rearrange("(k p) -> p k", p=P))
        b_proj_sb = const.tile([P, KH], F32, tag="b_proj_sb")
        nc.gpsimd.dma_start(b_proj_sb, t["b_proj"].rearrange("(k p) -> p k", p=P))
        b_mlp1_sb = const.tile([P, 32], F32, tag="b_mlp1_sb")
        nc.gpsimd.dma_start(b_mlp1_sb, t["b_mlp1"].rearrange("(k p) -> p k", p=P))
        b_mlp2_sb = const.tile([P, KH], F32, tag="b_mlp2_sb")
        nc.gpsimd.dma_start(b_mlp2_sb, t["b_mlp2"].rearrange("(k p) -> p k", p=P))

        # ---------------- input x + first w_ada half-blocks ----------------
        xall = xpool.tile([P, KH, S], BF16, tag="xall")
        nc.sync.dma_start(xall[:, 0:4, :], t["xT8"][:, 0:4, :])
        nc.sync.dma_start(xall[:, 4:8, :], t["xT8"][:, 4:8, :])

        # w_ada streamed as 12 half-blocks [P, KH, 512] (4 cmod cols each)
        def wada_dma(hb, eng=None):
            wt = wada_pool.tile([P, KH, 512], BF16, tag="wada")
            blk, mlo = hb // 2, (hb % 2) * 512
            (eng or nc.gpsimd).dma_start(
                wt, t["w_ada_t"][:, blk, :, mlo:mlo + 512])
            return wt

        wada_tiles = {hb: wada_dma(hb, nc.sync) for hb in range(2)}

        # ---------------- silu(c) ------------------------------------------
        sc_sb = const.tile([P, KH], BF16, tag="sc_sb")
        nc.scalar.activation(sc_sb, c_sb, AF.Silu)

        # attention SBUF tiles (allocated early for pool stack order)
        att_ctx = ExitStack()
        kq_pool = att_ctx.enter_context(tc.tile_pool(name="kq", bufs=1))
        k8 = [kq_pool.tile([P, 2, S], FP8, tag=f"k8_{i}", name=f"k8_{i}")
              for i in range(KS)]
        q8 = [kq_pool.tile([P, 2, S], FP8, tag=f"q8_{i}", name=f"q8_{i}")
              for i in range(KS)]
        for i in range(KS):
            nc.vector.memset(k8[i][:, 1, :].bitcast(mybir.dt.uint32), 0)
            nc.vector.memset(q8[i][:, 1, :].bitcast(mybir.dt.uint32), 0)
        v2 = [kq_pool.tile([P, 2, NH // 2, 2, CH + 32], FP8, tag=f"v2_{i}",
                           name=f"v2_{i}")
              for i in range(4)]
        for i in range(4):
            nc.vector.memset(v2[i][:, :, :, :, CH:CH + 32], 2.0)
        y8 = [kq_pool.tile([P, 2, S], FP8, tag=f"y8_{i}", name=f"y8_{i}")
              for i in range(4)]

        # ---------------- phase-1 psum pools --------------------------------
        ph1 = ExitStack()
        ps_ada = ph1.enter_context(tc.tile_pool(name="ps_ada", bufs=1, space="PSUM"))
        psada = ps_ada.tile([P, 48], F32, tag="ada")
        cmod = const.tile([P, 48], F32, tag="cmod")

        ln1_ps = ExitStack()
        ps_ln = ln1_ps.enter_context(tc.tile_pool(name="ps_ln", bufs=1, space="PSUM"))

        # ---------------- LN statistics (x stationary, ones moving) --------
        def ln_stats(pool, src, xsq_tag):
            """Returns psum [128, KS*KH] partial sums & sumsq (col tcv*KH+kc),
            token s = tc*128 + p. Each matmul is an independent start/stop
            group: interleaved accumulation in one PSUM bank is NOT safe (the
            start flag marks the whole 2 KiB bank pending-zero, wiping other
            columns' later accumulating writes), but completed columns' data
            survives subsequent starts."""
            pss = pool.tile([P, KS * KH], F32, tag="ln_s")
            psq = pool.tile([P, KS * KH], F32, tag="ln_q")
            for kc in range(KH):
                xsq = work.tile([P, S], BF16, tag=xsq_tag)
                nc.scalar.activation(xsq, src[:, kc, :], AF.Square)
                for tcv in range(KS):
                    sl = slice(tcv * P, (tcv + 1) * P)
                    col = tcv * KH + kc
                    nc.tensor.matmul(
                        pss[:, col:col + 1], lhsT=src[:, kc, sl], rhs=ones_mv,
                        start=True, stop=True,
                    )
                    nc.tensor.matmul(
                        psq[:, col:col + 1], lhsT=xsq[:, sl], rhs=ones_mv,
                        start=True, stop=True,
                    )
            return pss, psq

        def ln_finish(pss, psq, o2_base):
            """rstd & mean*rstd from [128, KS] stats; bf16 via scr2 to
            partition-broadcast tiles [128, S]."""
            pssum = rows.tile([P, KS], F32, tag="pssum")
            nc.vector.tensor_reduce(
                pssum, pss.rearrange("p (t k) -> p t k", t=KS),
                axis=mybir.AxisListType.X, op=OP.add)
            psqs = rows.tile([P, KS], F32, tag="psqs")
            nc.vector.tensor_reduce(
                psqs, psq.rearrange("p (t k) -> p t k", t=KS),
                axis=mybir.AxisListType.X, op=OP.add)
            m = rows.tile([P, KS], F32, tag="m_tok")
            nc.vector.tensor_scalar(out=m, in0=pssum, scalar1=1.0 / H,
                                    scalar2=0.0, op0=OP.mult, op1=OP.bypass)
            v = rows.tile([P, KS], F32, tag="v_tok")
            nc.vector.tensor_scalar(out=v, in0=psqs, scalar1=1.0 / H,
                                    scalar2=EPS, op0=OP.mult, op1=OP.add)
            msq = rows.tile([P, KS], F32, tag="msq_tok")
            nc.vector.tensor_tensor(msq, m, m, OP.mult)
            nc.vector.tensor_tensor(v, v, msq, OP.subtract)
            r = rows.tile([P, KS], F32, tag="r_tok")
            nc.vector.tensor_scalar(out=r, in0=v, scalar1=-0.5, scalar2=1.5,
                                    op0=OP.mult, op1=OP.add)
            s = rows.tile([P, KS], F32, tag="s_tok")
            for _ in range(2):
                nc.vector.tensor_tensor(s, r, r, OP.mult)
                nc.vector.tensor_tensor(s, s, v, OP.mult)
                nc.vector.tensor_scalar(out=s, in0=s, scalar1=-0.5, scalar2=1.5,
                                        op0=OP.mult, op1=OP.add)
                nc.vector.tensor_tensor(r, r, s, OP.mult)
            nc.vector.tensor_tensor(m, m, r, OP.mult)  # m <- m * r
            rb16 = rows.tile([P, KS], BF16, tag="rb16")
            nc.vector.tensor_copy(rb16, r)
            mb16 = rows.tile([P, KS], BF16, tag="mb16")
            nc.vector.tensor_copy(mb16, m)
            nc.gpsimd.dma_start(scr2_tok(o2_base, S), rb16)
            nc.gpsimd.dma_start(scr2_tok(o2_base + S, S), mb16)
            r_b = bcast.tile([P, S], BF16, tag="r_b")
            mr_b = bcast.tile([P, S], BF16, tag="mr_b")
            nc.gpsimd.dma_start(r_b, pbcast(scr2_row(o2_base, S), P))
            nc.gpsimd.dma_start(mr_b, pbcast(scr2_row(o2_base + S, S), P))
            return r_b, mr_b

        pss1, psq1 = ln_stats(ps_ln, xall, "xsq")

        # adaLN GEMV: one half-block = 4 cmod columns of 128
        def ada_halfblock(hb, wt, ps, base):
            for mcol in range(4):
                col = hb * 4 + mcol - base
                for kc in range(KH):
                    nc.tensor.matmul(
                        ps[:, col:col + 1],
                        lhsT=wt[:, kc, mcol * P:(mcol + 1) * P],
                        rhs=sc_sb[:, kc:kc + 1],
                        start=(kc == 0), stop=(kc == KH - 1),
                    )
            nc.vector.tensor_tensor(
                cmod[:, hb * 4:(hb + 1) * 4],
                ps[:, hb * 4 - base:(hb + 1) * 4 - base],
                b_ada_sb[:, hb * 4:(hb + 1) * 4], OP.add,
            )

        for hb in range(2):  # shift_msa
            ada_halfblock(hb, wada_tiles.pop(hb), psada, 0)

        r1_b, mr1_b = ln_finish(pss1, psq1, SCR2_LN)
        for hb in range(2, 4):  # scale_msa, behind the r/mr loads in the FIFO
            ada_halfblock(hb, wada_dma(hb), psada, 0)
        ln1_ps.close()

        sc1 = const.tile([P, 16], F32, tag="sc1")  # 1+scale_msa | 1+scale_mlp
        nc.scalar.add(sc1[:, 0:8], cmod[:, 8:16], 1.0)

        # ---------------- z1 modulate + qkv (DoubleRow fp8) ----------------
        zpool = ExitStack()
        z1_pool = zpool.enter_context(tc.tile_pool(name="z1", bufs=1))
        z1 = [z1_pool.tile([P, 2, S], FP8, tag=f"z1_{i}", name=f"z1_{i}")
              for i in range(4)]

        wqkv_ctx = ExitStack()
        wqkv_pool = wqkv_ctx.enter_context(tc.tile_pool(name="wqkv", bufs=1))
        wqkv = wqkv_pool.tile([P, 4, 2, 3 * H], FP8, tag="wqkv")
        for sec in range(3):
            nc.gpsimd.dma_start(
                wqkv[:, :, :, sec * H:(sec + 1) * H],
                t["w_qkv8"][:, :, :, sec * H:(sec + 1) * H],
            )

        def modulate(dst, src, r_b, mr_b, col, shift_ap, sl):
            tm = work.tile([P, S], BF16, tag="mod_tm")
            nc.vector.tensor_tensor(tm[:, sl], src[:, sl], r_b[:, sl], OP.mult)
            nc.vector.tensor_tensor(tm[:, sl], tm[:, sl], mr_b[:, sl],
                                    OP.subtract)
            nc.scalar.activation(dst[:, sl], tm[:, sl], AF.Identity,
                                 bias=shift_ap, scale=sc1[:, col:col + 1])

        for half in range(2):
            sl = slice(half * 512, (half + 1) * 512)
            for kc in range(KH):
                modulate(z1[kc // 2][:, kc % 2, :], xall[:, kc, :], r1_b, mr1_b,
                         kc, cmod[:, kc:kc + 1], sl)

        ps_mm_ctx = ExitStack()
        ps_mm = ps_mm_ctx.enter_context(
            tc.tile_pool(name="ps_mm", bufs=3, space="PSUM"))

        for oc in range(16):  # 8 k-chunks then 8 q-chunks
            ps = ps_mm.tile([P, S], F32, tag="mm")
            for half in range(2):
                sl = slice(half * 512, (half + 1) * 512)
                for kcp in range(4):
                    nc.tensor.matmul(
                        ps[:, sl],
                        lhsT=wqkv[:, kcp, :, oc * P:(oc + 1) * P],
                        rhs=z1[kcp][:, :, sl],
                        start=(kcp == 0), stop=(kcp == 3), perf_mode=DR,
                    )
            dst = k8 if oc < 8 else q8
            nc.scalar.copy(dst[oc % 8][:, 0, :], ps)

        for sc in range(KS):  # v, token-major
            ps = ps_mm.tile([P, S], F32, tag="mm")
            for half in range(2):
                sl = slice(2048 + half * 512, 2048 + (half + 1) * 512)
                osl = slice(half * 512, (half + 1) * 512)
                for kcp in range(4):
                    nc.tensor.matmul(
                        ps[:, osl],
                        lhsT=z1[kcp][:, :, sc * P:(sc + 1) * P],
                        rhs=wqkv[:, kcp, :, sl],
                        start=(kcp == 0), stop=(kcp == 3), perf_mode=DR,
                    )
            nc.scalar.copy(
                v2[sc // 2][:, sc % 2, :, :, 0:CH],
                ps.rearrange("p (hp two c) -> p hp two c", hp=NH // 2, two=2),
            )
        ps_mm_ctx.close()
        wqkv_ctx.close()
        zpool.close()
        ph1.close()

        # ---------------- attention ----------------------------------------
        wexp_pool = att_ctx.enter_context(tc.tile_pool(name="wexp", bufs=8))
        att_tmp = att_ctx.enter_context(tc.tile_pool(name="att_tmp", bufs=3))
        rdb_pool = att_ctx.enter_context(tc.tile_pool(name="rdb", bufs=3))
        wproj_pool = att_ctx.enter_context(tc.tile_pool(name="wproj", bufs=1))
        wproj = wproj_pool.tile([P, 4, 2, H], FP8, tag="wproj")

        att_ps = ExitStack()
        spool = att_ps.enter_context(tc.tile_pool(name="spool", bufs=2, space="PSUM"))
        avpool = att_ps.enter_context(tc.tile_pool(name="avpool", bufs=2, space="PSUM"))

        DVE_EXP_KC = (7,)

        def head_scores(h):
            ti, off = h // 2, 64 * (h % 2)
            prow = slice(off, off + CH)
            wexp = []
            for kcp in range(4):
                wt = wexp_pool.tile([P, 2, S], FP8, tag="wexp")
                for j in range(2):
                    kc = 2 * kcp + j
                    ps_s = spool.tile([P, S], F32, tag="ps")
                    for half in range(2):
                        sl = slice(half * 512, (half + 1) * 512)
                        nc.tensor.matmul(
                            ps_s[:, sl],
                            lhsT=k8[ti][prow, :, kc * P:(kc + 1) * P],
                            rhs=q8[ti][prow, :, sl],
                            start=True, stop=True, perf_mode=DR,
                        )
                    if kc in DVE_EXP_KC:
                        # exp(t) ~ 1 + t(1 + t/2), |t| < 0.5 (err < 1e-3)
                        tq = work.tile([P, S], BF16, tag="mod_tm")
                        nc.vector.tensor_scalar(
                            out=tq, in0=ps_s,
                            scalar1=1.0 / (64.0 * WS * WS), scalar2=0.0,
                            op0=OP.mult, op1=OP.bypass)
                        uq = work.tile([P, S], BF16, tag="res_tmp")
                        nc.vector.tensor_scalar(
                            out=uq, in0=tq, scalar1=0.5, scalar2=1.0,
                            op0=OP.mult, op1=OP.add)
                        nc.vector.tensor_tensor(uq, tq, uq, OP.mult)
                        nc.vector.tensor_scalar(
                            out=wt[:, j, :], in0=uq, scalar1=1.0, scalar2=1.0,
                            op0=OP.mult, op1=OP.add)
                    else:
                        nc.scalar.activation(wt[:, j, :], ps_s, AF.Exp,
                                             scale=1.0 / (64.0 * WS * WS))
                wexp.append(wt)
            return wexp

        def head_av(h, wexp):
            """AV matmul with the softmax denominator fused in: the
            stationary operand is [v_head | 32 ones-cols valued 2.0], so
            output rows 0:64 are y_unnorm and rows 64:96 are 2*den — one
            DoubleRow group at base partition 0 (ISA-safe), no extra cost
            (matmul cost is output free size only)."""
            ps_y = avpool.tile([P, S], F32, tag="ps_y")
            for half in range(2):
                sl = slice(half * 512, (half + 1) * 512)
                for kcp in range(4):
                    nc.tensor.matmul(
                        ps_y[0:CH + 32, sl],
                        lhsT=v2[kcp][:, :, h // 2, h % 2, :],
                        rhs=wexp[kcp][:, :, sl],
                        start=(kcp == 0), stop=(kcp == 3), perf_mode=DR,
                    )
            drow = att_tmp.tile([1, S], BF16, tag="drow", bufs=2)
            with nc.allow_low_precision(reason="softmax 1/den in bf16"):
                nc.vector.reciprocal(drow, ps_y[CH:CH + 1, :])  # 0.5/den
            o2 = SCR2_HEAD + h * S
            nc.gpsimd.dma_start(scr2_row(o2, S), drow)
            rdb = rdb_pool.tile([P, S], BF16, tag="rdb")
            nc.gpsimd.dma_start(rdb, pbcast(scr2_row(o2, S), P))
            return ps_y, rdb

        def head_norm(h, ps_y, rdb):
            ti, j, off = h // 4, (h % 4) // 2, 64 * (h % 2)
            nc.vector.tensor_tensor(
                y8[ti][off:off + CH, j, :],
                ps_y[0:CH, :], rdb[0:CH, :], OP.mult,
            )

        def late_streams(step):
            if step == 2:
                nc.gpsimd.dma_start(wproj, t["w_proj8"])
            elif 4 <= step < 12:  # w_ada blocks 4..11
                wada_tiles[step] = wada_dma(step)
            elif 12 <= step < 20:  # w_mlp1 hi, 1MB pieces
                i = step - 12
                nc.gpsimd.dma_start(wmlp1[:, i // 2, :, (i % 2) * 2048:
                                          (i % 2) * 2048 + 2048],
                                    t["w_mlp18"][:, i // 2, :, (i % 2) * 2048:
                                                 (i % 2) * 2048 + 2048])
            elif 20 <= step < 28:  # w_mlp1 lo
                i = step - 20
                nc.gpsimd.dma_start(wmlp1l[:, i // 2, :, (i % 2) * 2048:
                                           (i % 2) * 2048 + 2048],
                                    t["w_mlp18l"][:, i // 2, :, (i % 2) * 2048:
                                                  (i % 2) * 2048 + 2048])

            if 6 <= step < 14:  # adaLN tail rides the scores psum ring
                hb = step - 2
                psx = spool.tile([P, S], F32, tag="ps")
                ada_halfblock(hb, wada_tiles.pop(hb), psx, hb * 4)

        st = {}
        for step in range(28):
            late_streams(step)
            if step < NH:
                st[step] = {"wexp": head_scores(step)}
            if 1 <= step and step - 1 < NH:
                hh = step - 1
                ps_y, rdb = head_av(hh, st[hh].pop("wexp"))
                st[hh]["ps_y"], st[hh]["rdb"] = ps_y, rdb
            if 2 <= step and step - 2 < NH:
                hh = step - 2
                head_norm(hh, st[hh].pop("ps_y"), st[hh].pop("rdb"))
                del st[hh]
        att_ps.close()


        nc.scalar.add(sc1[:, 8:16], cmod[:, 32:40], 1.0)
        gpr = const.tile([P, KH], F32, tag="gpr")
        nc.vector.tensor_scalar(out=gpr, in0=cmod[:, 16:24],
                                scalar1=1.0 / 512.0, scalar2=0.0,
                                op0=OP.mult, op1=OP.bypass)
        gpb = const.tile([P, KH], F32, tag="gpb")
        nc.vector.tensor_tensor(gpb, cmod[:, 16:24], b_proj_sb, OP.mult)
        gmr = const.tile([P, KH], F32, tag="gmr")
        nc.vector.tensor_scalar(out=gmr, in0=cmod[:, 40:48],
                                scalar1=1.0 / 32.0, scalar2=0.0,
                                op0=OP.mult, op1=OP.bypass)
        gmb = const.tile([P, KH], F32, tag="gmb")
        nc.vector.tensor_tensor(gmb, cmod[:, 40:48], b_mlp2_sb, OP.mult)

        # ---------------- proj + gated residual + LN2 stats -----------------
        ph3 = ExitStack()
        ps_pr = ph3.enter_context(tc.tile_pool(name="ps_pr", bufs=2, space="PSUM"))
        ps_ln2 = ph3.enter_context(tc.tile_pool(name="ps_ln2", bufs=1, space="PSUM"))
        pss2 = ps_ln2.tile([P, KS * KH], F32, tag="ln_s")
        psq2 = ps_ln2.tile([P, KS * KH], F32, tag="ln_q")
        for mc in range(KH):
            ps = ps_pr.tile([P, S], F32, tag="mm")
            for half in range(2):
                sl = slice(half * 512, (half + 1) * 512)
                for ti in range(4):
                    nc.tensor.matmul(
                        ps[:, sl],
                        lhsT=wproj[:, ti, :, mc * P:(mc + 1) * P],
                        rhs=y8[ti][:, :, sl],
                        start=(ti == 0), stop=(ti == 3), perf_mode=DR,
                    )
            tp = work.tile([P, S], BF16, tag="res_tmp")
            nc.scalar.activation(tp, ps, AF.Identity,
                                 bias=gpb[:, mc:mc + 1],
                                 scale=gpr[:, mc:mc + 1])
            nc.vector.tensor_tensor(xall[:, mc, :], xall[:, mc, :], tp, OP.add)
            # LN2 statistics for this chunk right away
            xsq = work.tile([P, S], BF16, tag="xsq")
            nc.vector.tensor_tensor(xsq, xall[:, mc, :], xall[:, mc, :],
                                    OP.mult)
            for tcv in range(KS):
                sl = slice(tcv * P, (tcv + 1) * P)
                col = tcv * KH + mc
                nc.tensor.matmul(
                    pss2[:, col:col + 1], lhsT=xall[:, mc, sl], rhs=ones_mv,
                    start=True, stop=True,
                )
                nc.tensor.matmul(
                    psq2[:, col:col + 1], lhsT=xsq[:, sl], rhs=ones_mv,
                    start=True, stop=True,
                )
        att_ctx.close()

        # ---------------- LN2 finish + modulate z2 + MLP --------------------
        ph4 = ExitStack()
        r2_b, mr2_b = ln_finish(pss2, psq2, SCR2_LN + 2 * S)
        ph3.close()
        wada_ctx.close()

        h_pool = ph4.enter_context(tc.tile_pool(name="h8", bufs=1))
        h8 = [h_pool.tile([P, 2, S], FP8, tag=f"h8_{i}", name=f"h8_{i}")
              for i in range(16)]
        z2_pool = ph4.enter_context(tc.tile_pool(name="z2", bufs=1))
        z2 = [z2_pool.tile([P, 2, S], FP8, tag=f"z2_{i}", name=f"z2_{i}")
              for i in range(4)]
        z2l = [z2_pool.tile([P, 2, S], FP8, tag=f"z2l_{i}", name=f"z2l_{i}")
               for i in range(4)]
        for half in range(2):
            sl = slice(half * 512, (half + 1) * 512)
            for kc in range(KH):
                # z_bf (bf16) -> z_hi (fp8) -> z_lo = fp8(z_bf - z_hi)
                tm = work.tile([P, S], BF16, tag="mod_tm")
                nc.vector.tensor_tensor(tm[:, sl], xall[:, kc, sl],
                                        r2_b[:, sl], OP.mult)
                nc.vector.tensor_tensor(tm[:, sl], tm[:, sl], mr2_b[:, sl],
                                        OP.subtract)
                zbf = work.tile([P, S], BF16, tag="stage_bf")
                nc.vector.tensor_scalar(
                    out=zbf[:, sl], in0=tm[:, sl],
                    scalar1=sc1[:, 8 + kc:8 + kc + 1],
                    scalar2=cmod[:, 24 + kc:24 + kc + 1],
                    op0=OP.mult, op1=OP.add,
                )
                zhi = z2[kc // 2][:, kc % 2, :]
                nc.scalar.copy(zhi[:, sl], zbf[:, sl])
                nc.vector.tensor_tensor(z2l[kc // 2][:, kc % 2, sl],
                                        zbf[:, sl], zhi[:, sl], OP.subtract)

        wmlp2_pool = ph4.enter_context(tc.tile_pool(name="wmlp2", bufs=2))
        otmp_pool = ph4.enter_context(tc.tile_pool(name="otmp", bufs=2))

        def w2_blk_dma(mc):  # 1 out-chunk of hi+lo
            bh = wmlp2_pool.tile([P, 16, 2, P], FP8, tag="w2hi")
            nc.sync.dma_start(bh, t["w_mlp28"][:, :, :, mc * P:(mc + 1) * P])
            bl = wmlp2_pool.tile([P, 16, 2, P], FP8, tag="w2lo")
            nc.sync.dma_start(bl, t["w_mlp28l"][:, :, :, mc * P:(mc + 1) * P])
            return bh, bl

        w2blk = {0: w2_blk_dma(0), 1: w2_blk_dma(1)}

        m1_ctx = ExitStack()
        ps_m1 = m1_ctx.enter_context(tc.tile_pool(name="ps_m1", bufs=3, space="PSUM"))

        for mc in range(32):
            ps = ps_m1.tile([P, S], F32, tag="mm")
            for half in range(2):
                sl = slice(half * 512, (half + 1) * 512)
                for p_ in range(3):
                    wsrc = wmlp1 if p_ != 1 else wmlp1l
                    zsrc = z2 if p_ != 2 else z2l
                    for kcp in range(4):
                        nc.tensor.matmul(
                            ps[:, sl],
                            lhsT=wsrc[:, kcp, :, mc * P:(mc + 1) * P],
                            rhs=zsrc[kcp][:, :, sl],
                            start=(p_ == 0 and kcp == 0),
                            stop=(p_ == 2 and kcp == 3), perf_mode=DR,
                        )
            nc.scalar.activation(
                h8[mc // 2][:, mc % 2, :], ps, AF.Gelu_apprx_tanh,
                bias=b_mlp1_sb[:, mc:mc + 1], scale=1.0 / WS,
            )
        m1_ctx.close()

        w2_ctx = ExitStack()
        ps_m2 = w2_ctx.enter_context(tc.tile_pool(name="ps_m2", bufs=3, space="PSUM"))
        for mc in range(KH):
            if mc + 2 < KH:
                w2blk[mc + 2] = w2_blk_dma(mc + 2)
            bh, bl = w2blk.pop(mc)
            ps = ps_m2.tile([P, S], F32, tag="mm")
            off = 0
            for half in range(2):
                sl = slice(half * 512, (half + 1) * 512)
                for p_ in range(2):
                    wsrc = bh if p_ != 1 else bl
                    for kcp in range(16):
                        nc.tensor.matmul(
                            ps[:, sl],
                            lhsT=wsrc[:, kcp, :, off:off + P],
                            rhs=h8[kcp][:, :, sl],
                            start=(p_ == 0 and kcp == 0),
                            stop=(p_ == 1 and kcp == 15), perf_mode=DR,
                        )
            tp = work.tile([P, S], BF16, tag="res_tmp")
            nc.scalar.activation(tp, ps, AF.Identity,
                                 bias=gmb[:, mc:mc + 1],
                                 scale=gmr[:, mc:mc + 1])
            ot = otmp_pool.tile([P, S], F32, tag="ot")
            nc.vector.tensor_tensor(ot, xall[:, mc, :], tp, OP.add)
            nc.sync.dma_start(t["outT8"][:, mc, :], ot)
        w2_ctx.close()
        ph4.close()


@functools.lru_cache(maxsize=1)
def _get_nc():
    return _build_program()


def _fp8(a):
    return np.ascontiguousarray(
        np.clip(np.asarray(a, dtype=np.float32), -240.0, 240.0)
        .astype(ml_dtypes.float8_e4m3))


def kernel(x, c, w_ada, b_ada, w_qkv, w_proj, b_proj, w_mlp1, b_mlp1,
           w_mlp2, b_mlp2):
    nc = _get_nc()
    bf = ml_dtypes.bfloat16
    f32 = np.float32

    p = np.arange(128)
    w_qkv8 = _fp8((np.asarray(w_qkv, f32) * WS)
                  .reshape(4, 2, 128, 3 * H).transpose(2, 0, 1, 3))

    # --- w_proj row permutation matching the y8 layout ---
    phi = np.empty((128, 4, 2), np.int64)
    for ti in range(4):
        for j in range(2):
            phi[:, ti, j] = (4 * ti + 2 * j + p // 64) * CH + (p % 64)
    w_proj8 = _fp8((np.asarray(w_proj, f32) * WS)[phi])

    w1s = (np.asarray(w_mlp1, f32) * WS).reshape(4, 2, 128, 4 * H)\
        .transpose(2, 0, 1, 3)
    w_mlp18 = _fp8(w1s)
    w_mlp18l = _fp8(w1s - w_mlp18.astype(f32))
    w2s = (np.asarray(w_mlp2, f32) * WS).reshape(16, 2, 128, H)\
        .transpose(2, 0, 1, 3)
    w_mlp28 = _fp8(w2s)
    w_mlp28l = _fp8(w2s - w_mlp28.astype(f32))
    w_ada_t = np.ascontiguousarray(
        np.asarray(w_ada, f32).reshape(8, 128, 6, 1024)
        .transpose(1, 2, 0, 3).astype(bf))

    shared = {
        "w_ada_t": w_ada_t,
        "b_ada": np.ascontiguousarray(b_ada, dtype=f32),
        "w_qkv8": w_qkv8,
        "w_proj8": w_proj8,
        "b_proj": np.ascontiguousarray(b_proj, dtype=f32),
        "w_mlp18": w_mlp18,
        "w_mlp18l": w_mlp18l,
        "b_mlp1": np.ascontiguousarray(b_mlp1, dtype=f32),
        "w_mlp28": w_mlp28,
        "w_mlp28l": w_mlp28l,
        "b_mlp2": np.ascontiguousarray(b_mlp2, dtype=f32),
    }
    in_maps = []
    for bidx in range(N_CORES):
        m = dict(shared)
        m["xT8"] = np.ascontiguousarray(
            np.asarray(x[bidx], f32).T.reshape(8, 128, S)
            .transpose(1, 0, 2).astype(bf))
        m["cvec"] = np.ascontiguousarray(np.asarray(c[bidx], dtype=f32))
        in_maps.append(m)

    res = run_bass_kernel_spmd(
        nc, in_maps, core_ids=list(range(N_CORES)), trace=False
    )
    kernel.last_results = res

    out = np.empty((B, S, H), dtype=f32)
    for bidx in range(N_CORES):
        o = np.asarray(res.results[bidx]["outT8"])  # [128, 8, S]
        out[bidx] = o.transpose(1, 0, 2).reshape(H, S).T
    return out


if __name__ == "__main__":
    nc = _get_nc()
    print("program built ok")



# revision 50
# speedup vs baseline: 1.0851x; 1.0166x over previous
"""DiT block kernel for Trainium2, data-parallel over batch (8 cores, B=8).

v2: fp8 DoubleRow matmuls for qkv/scores/AV/proj/mlp1/mlp2 (2 K-chunks per
pass), LN statistics and adaLN GEMV computed with x/w_ada as the *stationary*
operand and a tiny moving operand (cost ~ output free size), per-token
quantities (LN rstd, softmax 1/den) produced directly in token-on-partition
layout via PE transposes of single rows, elementwise work spread across
DVE / ACT / GPSIMD.

Layouts (host-retiled):
  xT8      [128, 8, 1024]  bf16  x[b].T tiled: [p, kc, s] = x[b, s, kc*128+p]
  w_qkv8   [128, 4, 2, 3072] fp8 (x32), k/q column blocks permuted so that
           head h lives on partitions 32*(h%4).. with c split across the
           DoubleRow slot dim (c%32 on partitions, c//32 on slot)
  w_proj8  [128, 4, 2, 1024] fp8 (x32), rows permuted to match the y8 layout
           head h -> tile h//4, slot (h%4)//2, partition base 64*(h%2)
  w_mlp18  [128, 4, 2, 4096] fp8 (x32)
  w_mlp28  [128, 16, 2, 1024] fp8 (x32)
  w_ada_t  [128, 6, 8, 1024] bf16  [p, blk, kc, m] = w_ada[kc*128+p, blk*1024+m]
  outT8    [128, 8, 1024]  f32   [p, mc, s] = out[b, s, mc*128+p]

Scale bookkeeping: weights x32 in fp8. scoresT psum = 1024*k.q -> exp scale
1/(64*1024). AV y8 = 16*y_true (rd = 0.5/den). proj psum = 512*attn -> gate/512.
mlp1 psum = 32*pre -> gelu scale 1/32. mlp2 psum = 32*mlp -> gate/32.
"""

import os
import sys
import functools
from contextlib import ExitStack

import numpy as np

for _p in ("/opt/trn_rl_repo", "/root/.axon_site/_ro/trn_rl_repo"):
    if os.path.isdir(_p) and _p not in sys.path:
        sys.path.insert(0, _p)

import ml_dtypes  # noqa: E402
import concourse.bass as bass  # noqa: E402
from concourse import bacc  # noqa: E402
import concourse.tile as tile  # noqa: E402
from concourse import mybir  # noqa: E402
from concourse.bass_utils import run_bass_kernel_spmd  # noqa: E402

F32 = mybir.dt.float32
BF16 = mybir.dt.bfloat16
FP8 = mybir.dt.float8e4
AF = mybir.ActivationFunctionType
OP = mybir.AluOpType
DR = mybir.MatmulPerfMode.DoubleRow

B, S, H, NH, CH = 8, 1024, 1024, 16, 64
P = 128
KH = H // P          # 8 chunks over H
KS = S // P          # 8 chunks over S
EPS = 1e-6
N_CORES = 8
WS = 32.0            # fp8 weight scale

# scr2 (bf16) scratch layout: LN r/mr (2 LNs x 2 rows x S) then per-head rd
SCR2_LN = 0
SCR2_HEAD = 4 * S
SCR2_N = SCR2_HEAD + NH * S


def _build_program():
    nc = bacc.Bacc("TRN2", target_bir_lowering=False, debug=False)

    t = {}
    t["xT8"] = nc.dram_tensor("xT8", (P, KH, S), BF16, kind="ExternalInput").ap()
    t["cvec"] = nc.dram_tensor("cvec", (H,), F32, kind="ExternalInput").ap()
    t["w_ada_t"] = nc.dram_tensor("w_ada_t", (P, 6, KH, 1024), BF16,
                                  kind="ExternalInput").ap()
    t["b_ada"] = nc.dram_tensor("b_ada", (6 * H,), F32, kind="ExternalInput").ap()
    t["w_qkv8"] = nc.dram_tensor("w_qkv8", (P, 4, 2, 3 * H), FP8,
                                 kind="ExternalInput").ap()
    t["w_proj8"] = nc.dram_tensor("w_proj8", (P, 4, 2, H), FP8,
                                  kind="ExternalInput").ap()
    t["b_proj"] = nc.dram_tensor("b_proj", (H,), F32, kind="ExternalInput").ap()
    t["w_mlp18"] = nc.dram_tensor("w_mlp18", (P, 4, 2, 4 * H), FP8,
                                  kind="ExternalInput").ap()
    t["w_mlp18l"] = nc.dram_tensor("w_mlp18l", (P, 4, 2, 4 * H), FP8,
                                   kind="ExternalInput").ap()
    t["b_mlp1"] = nc.dram_tensor("b_mlp1", (4 * H,), F32, kind="ExternalInput").ap()
    t["w_mlp28"] = nc.dram_tensor("w_mlp28", (P, 16, 2, H), FP8,
                                  kind="ExternalInput").ap()
    t["w_mlp28l"] = nc.dram_tensor("w_mlp28l", (P, 16, 2, H), FP8,
                                   kind="ExternalInput").ap()
    t["b_mlp2"] = nc.dram_tensor("b_mlp2", (H,), F32, kind="ExternalInput").ap()
    t["outT8"] = nc.dram_tensor("outT8", (P, KH, S), F32, kind="ExternalOutput").ap()
    t["scr2"] = nc.dram_tensor("scr2", (SCR2_N,), BF16, kind="ExternalOutput").ap()

    with tile.TileContext(nc) as tc:
        _emit(tc, t)
    nc.compile()
    return nc


def _emit(tc, t):
    nc = tc.nc
    scr2 = t["scr2"]

    def pbcast(ap_1p, nparts):
        """Partition-broadcast view of a 1-partition (DRAM) AP."""
        return bass.AP(
            tensor=ap_1p.tensor, offset=ap_1p.offset,
            ap=[[0, nparts]] + list(ap_1p.ap[1:]),
        )

    def scr2_row(off, n):
        return scr2[off:off + n].rearrange("(a n) -> a n", a=1)

    def scr2_tok(off, n):
        """[128, n//128] view; flat[k*128+p] = element [p, k]."""
        return scr2[off:off + n].rearrange("(k p) -> p k", p=P)

    with ExitStack() as ctx:
        const = ctx.enter_context(tc.tile_pool(name="const", bufs=1))
        rows = ctx.enter_context(tc.tile_pool(name="rows", bufs=1))
        work = ctx.enter_context(tc.tile_pool(name="work", bufs=2))
        xpool = ctx.enter_context(tc.tile_pool(name="xpool", bufs=1))
        bcast = ctx.enter_context(tc.tile_pool(name="bcast", bufs=1))
        wmlp1_pool = ctx.enter_context(tc.tile_pool(name="wmlp1", bufs=1))
        wmlp1 = wmlp1_pool.tile([P, 4, 2, 4 * H], FP8, tag="wmlp1")
        wmlp1l = wmlp1_pool.tile([P, 4, 2, 4 * H], FP8, tag="wmlp1l")
        wada_ctx = ExitStack()
        wada_pool = wada_ctx.enter_context(tc.tile_pool(name="wada", bufs=1))

        # ---------------- constants ----------------------------------------
        ones_mv = const.tile([P, 1], BF16, tag="ones_mv")
        nc.vector.memset(ones_mv, 1.0)

        c_sb = const.tile([P, KH], F32, tag="c_sb")
        nc.gpsimd.dma_start(c_sb, t["cvec"].rearrange("(k p) -> p k", p=P))
        b_ada_sb = const.tile([P, 48], F32, tag="b_ada_sb")
        nc.gpsimd.dma_start(b_ada_sb, t["b_ada"].rearrange("(k p) -> p k", p=P))
        b_proj_sb = const.tile([P, KH], F32, tag="b_proj_sb")
        nc.gpsimd.dma_start(b_proj_sb, t["b_proj"].rearrange("(k p) -> p k", p=P))
        b_mlp1_sb = const.tile([P, 32], F32, tag="b_mlp1_sb")
        nc.gpsimd.dma_start(b_mlp1_sb, t["b_mlp1"].rearrange("(k p) -> p k", p=P))
        b_mlp2_sb = const.tile([P, KH], F32, tag="b_mlp2_sb")
        nc.gpsimd.dma_start(b_mlp2_sb, t["b_mlp2"].rearrange("(k p) -> p k", p=P))

        # ---------------- input x + first w_ada half-blocks ----------------
        xall = xpool.tile([P, KH, S], BF16, tag="xall")
        nc.sync.dma_start(xall[:, 0:4, :], t["xT8"][:, 0:4, :])
        nc.sync.dma_start(xall[:, 4:8, :], t["xT8"][:, 4:8, :])

        # w_ada streamed as 12 half-blocks [P, KH, 512] (4 cmod cols each)
        def wada_dma(hb, eng=None):
            wt = wada_pool.tile([P, KH, 512], BF16, tag="wada")
            blk, mlo = hb // 2, (hb % 2) * 512
            (eng or nc.gpsimd).dma_start(
                wt, t["w_ada_t"][:, blk, :, mlo:mlo + 512])
            return wt

        wada_tiles = {hb: wada_dma(hb, nc.sync) for hb in range(2)}

        # ---------------- silu(c) ------------------------------------------
        sc_sb = const.tile([P, KH], BF16, tag="sc_sb")
        nc.scalar.activation(sc_sb, c_sb, AF.Silu)

        # attention SBUF tiles (allocated early for pool stack order)
        att_ctx = ExitStack()
        kq_pool = att_ctx.enter_context(tc.tile_pool(name="kq", bufs=1))
        k8 = [kq_pool.tile([P, 2, S], FP8, tag=f"k8_{i}", name=f"k8_{i}")
              for i in range(KS)]
        q8 = [kq_pool.tile([P, 2, S], FP8, tag=f"q8_{i}", name=f"q8_{i}")
              for i in range(KS)]
        for i in range(KS):
            nc.vector.memset(k8[i][:, 1, :].bitcast(mybir.dt.uint32), 0)
            nc.vector.memset(q8[i][:, 1, :].bitcast(mybir.dt.uint32), 0)
        v2 = [kq_pool.tile([P, 2, NH // 2, 2, CH + 32], FP8, tag=f"v2_{i}",
                           name=f"v2_{i}")
              for i in range(4)]
        for i in range(4):
            nc.vector.memset(v2[i][:, :, :, :, CH:CH + 32], 2.0)
        y8 = [kq_pool.tile([P, 2, S], FP8, tag=f"y8_{i}", name=f"y8_{i}")
              for i in range(4)]

        # ---------------- phase-1 psum pools --------------------------------
        ph1 = ExitStack()
        ps_ada = ph1.enter_context(tc.tile_pool(name="ps_ada", bufs=1, space="PSUM"))
        psada = ps_ada.tile([P, 48], F32, tag="ada")
        cmod = const.tile([P, 48], F32, tag="cmod")

        ln1_ps = ExitStack()
        ps_ln = ln1_ps.enter_context(tc.tile_pool(name="ps_ln", bufs=1, space="PSUM"))

        # ---------------- LN statistics (x stationary, ones moving) --------
        def ln_stats(pool, src, xsq_tag):
            """Returns psum [128, KS*KH] partial sums & sumsq (col tcv*KH+kc),
            token s = tc*128 + p. Each matmul is an independent start/stop
            group: interleaved accumulation in one PSUM bank is NOT safe (the
            start flag marks the whole 2 KiB bank pending-zero, wiping other
            columns' later accumulating writes), but completed columns' data
            survives subsequent starts."""
            pss = pool.tile([P, KS * KH], F32, tag="ln_s")
            psq = pool.tile([P, KS * KH], F32, tag="ln_q")
            for kc in range(KH):
                xsq = work.tile([P, S], BF16, tag=xsq_tag)
                nc.scalar.activation(xsq, src[:, kc, :], AF.Square)
                for tcv in range(KS):
                    sl = slice(tcv * P, (tcv + 1) * P)
                    col = tcv * KH + kc
                    nc.tensor.matmul(
                        pss[:, col:col + 1], lhsT=src[:, kc, sl], rhs=ones_mv,
                        start=True, stop=True,
                    )
                    nc.tensor.matmul(
                        psq[:, col:col + 1], lhsT=xsq[:, sl], rhs=ones_mv,
                        start=True, stop=True,
                    )
            return pss, psq

        def ln_finish(pss, psq, o2_base):
            """rstd & mean*rstd from [128, KS] stats; bf16 via scr2 to
            partition-broadcast tiles [128, S]."""
            pssum = rows.tile([P, KS], F32, tag="pssum")
            nc.vector.tensor_reduce(
                pssum, pss.rearrange("p (t k) -> p t k", t=KS),
                axis=mybir.AxisListType.X, op=OP.add)
            psqs = rows.tile([P, KS], F32, tag="psqs")
            nc.vector.tensor_reduce(
                psqs, psq.rearrange("p (t k) -> p t k", t=KS),
                axis=mybir.AxisListType.X, op=OP.add)
            m = rows.tile([P, KS], F32, tag="m_tok")
            nc.vector.tensor_scalar(out=m, in0=pssum, scalar1=1.0 / H,
                                    scalar2=0.0, op0=OP.mult, op1=OP.bypass)
            v = rows.tile([P, KS], F32, tag="v_tok")
            nc.vector.tensor_scalar(out=v, in0=psqs, scalar1=1.0 / H,
                                    scalar2=EPS, op0=OP.mult, op1=OP.add)
            msq = rows.tile([P, KS], F32, tag="msq_tok")
            nc.vector.tensor_tensor(msq, m, m, OP.mult)
            nc.vector.tensor_tensor(v, v, msq, OP.subtract)
            r = rows.tile([P, KS], F32, tag="r_tok")
            nc.vector.tensor_scalar(out=r, in0=v, scalar1=-0.5, scalar2=1.5,
                                    op0=OP.mult, op1=OP.add)
            s = rows.tile([P, KS], F32, tag="s_tok")
            for _ in range(2):
                nc.vector.tensor_tensor(s, r, r, OP.mult)
                nc.vector.tensor_tensor(s, s, v, OP.mult)
                nc.vector.tensor_scalar(out=s, in0=s, scalar1=-0.5, scalar2=1.5,
                                        op0=OP.mult, op1=OP.add)
                nc.vector.tensor_tensor(r, r, s, OP.mult)
            nc.vector.tensor_tensor(m, m, r, OP.mult)  # m <- m * r
            rb16 = rows.tile([P, KS], BF16, tag="rb16")
            nc.vector.tensor_copy(rb16, r)
            mb16 = rows.tile([P, KS], BF16, tag="mb16")
            nc.vector.tensor_copy(mb16, m)
            nc.gpsimd.dma_start(scr2_tok(o2_base, S), rb16)
            nc.gpsimd.dma_start(scr2_tok(o2_base + S, S), mb16)
            rrow = rows.tile([1, S], BF16, tag="rrow")
            mrow = rows.tile([1, S], BF16, tag="mrow")
            nc.sync.dma_start(rrow, scr2_row(o2_base, S))
            nc.sync.dma_start(mrow, scr2_row(o2_base + S, S))
            r_b = bcast.tile([P, S], BF16, tag="r_b")
            mr_b = bcast.tile([P, S], BF16, tag="mr_b")
            nc.gpsimd.partition_broadcast(r_b, rrow)
            nc.gpsimd.partition_broadcast(mr_b, mrow)
            return r_b, mr_b

        pss1, psq1 = ln_stats(ps_ln, xall, "xsq")

        # adaLN GEMV: one half-block = 4 cmod columns of 128
        def ada_halfblock(hb, wt, ps, base):
            for mcol in range(4):
                col = hb * 4 + mcol - base
                for kc in range(KH):
                    nc.tensor.matmul(
                        ps[:, col:col + 1],
                        lhsT=wt[:, kc, mcol * P:(mcol + 1) * P],
                        rhs=sc_sb[:, kc:kc + 1],
                        start=(kc == 0), stop=(kc == KH - 1),
                    )
            nc.vector.tensor_tensor(
                cmod[:, hb * 4:(hb + 1) * 4],
                ps[:, hb * 4 - base:(hb + 1) * 4 - base],
                b_ada_sb[:, hb * 4:(hb + 1) * 4], OP.add,
            )

        for hb in range(2):  # shift_msa
            ada_halfblock(hb, wada_tiles.pop(hb), psada, 0)

        r1_b, mr1_b = ln_finish(pss1, psq1, SCR2_LN)
        for hb in range(2, 4):  # scale_msa, behind the r/mr loads in the FIFO
            ada_halfblock(hb, wada_dma(hb), psada, 0)
        ln1_ps.close()

        sc1 = const.tile([P, 16], F32, tag="sc1")  # 1+scale_msa | 1+scale_mlp
        nc.scalar.add(sc1[:, 0:8], cmod[:, 8:16], 1.0)

        # ---------------- z1 modulate + qkv (DoubleRow fp8) ----------------
        zpool = ExitStack()
        z1_pool = zpool.enter_context(tc.tile_pool(name="z1", bufs=1))
        z1 = [z1_pool.tile([P, 2, S], FP8, tag=f"z1_{i}", name=f"z1_{i}")
              for i in range(4)]

        wqkv_ctx = ExitStack()
        wqkv_pool = wqkv_ctx.enter_context(tc.tile_pool(name="wqkv", bufs=1))
        wqkv = wqkv_pool.tile([P, 4, 2, 3 * H], FP8, tag="wqkv")
        for sec in range(3):
            nc.gpsimd.dma_start(
                wqkv[:, :, :, sec * H:(sec + 1) * H],
                t["w_qkv8"][:, :, :, sec * H:(sec + 1) * H],
            )

        def modulate(dst, src, r_b, mr_b, col, shift_ap, sl):
            tm = work.tile([P, S], BF16, tag="mod_tm")
            nc.vector.tensor_tensor(tm[:, sl], src[:, sl], r_b[:, sl], OP.mult)
            nc.vector.tensor_tensor(tm[:, sl], tm[:, sl], mr_b[:, sl],
                                    OP.subtract)
            nc.scalar.activation(dst[:, sl], tm[:, sl], AF.Identity,
                                 bias=shift_ap, scale=sc1[:, col:col + 1])

        for half in range(2):
            sl = slice(half * 512, (half + 1) * 512)
            for kc in range(KH):
                modulate(z1[kc // 2][:, kc % 2, :], xall[:, kc, :], r1_b, mr1_b,
                         kc, cmod[:, kc:kc + 1], sl)

        ps_mm_ctx = ExitStack()
        ps_mm = ps_mm_ctx.enter_context(
            tc.tile_pool(name="ps_mm", bufs=3, space="PSUM"))

        for oc in range(16):  # 8 k-chunks then 8 q-chunks
            ps = ps_mm.tile([P, S], F32, tag="mm")
            for half in range(2):
                sl = slice(half * 512, (half + 1) * 512)
                for kcp in range(4):
                    nc.tensor.matmul(
                        ps[:, sl],
                        lhsT=wqkv[:, kcp, :, oc * P:(oc + 1) * P],
                        rhs=z1[kcp][:, :, sl],
                        start=(kcp == 0), stop=(kcp == 3), perf_mode=DR,
                    )
            dst = k8 if oc < 8 else q8
            nc.scalar.copy(dst[oc % 8][:, 0, :], ps)

        for sc in range(KS):  # v, token-major
            ps = ps_mm.tile([P, S], F32, tag="mm")
            for half in range(2):
                sl = slice(2048 + half * 512, 2048 + (half + 1) * 512)
                osl = slice(half * 512, (half + 1) * 512)
                for kcp in range(4):
                    nc.tensor.matmul(
                        ps[:, osl],
                        lhsT=z1[kcp][:, :, sc * P:(sc + 1) * P],
                        rhs=wqkv[:, kcp, :, sl],
                        start=(kcp == 0), stop=(kcp == 3), perf_mode=DR,
                    )
            nc.scalar.copy(
                v2[sc // 2][:, sc % 2, :, :, 0:CH],
                ps.rearrange("p (hp two c) -> p hp two c", hp=NH // 2, two=2),
            )
        ps_mm_ctx.close()
        wqkv_ctx.close()
        zpool.close()
        ph1.close()

        # ---------------- attention ----------------------------------------
        wexp_pool = att_ctx.enter_context(tc.tile_pool(name="wexp", bufs=8))
        att_tmp = att_ctx.enter_context(tc.tile_pool(name="att_tmp", bufs=3))
        rdb_pool = att_ctx.enter_context(tc.tile_pool(name="rdb", bufs=3))
        wproj_pool = att_ctx.enter_context(tc.tile_pool(name="wproj", bufs=1))
        wproj = wproj_pool.tile([P, 4, 2, H], FP8, tag="wproj")

        att_ps = ExitStack()
        spool = att_ps.enter_context(tc.tile_pool(name="spool", bufs=2, space="PSUM"))
        avpool = att_ps.enter_context(tc.tile_pool(name="avpool", bufs=2, space="PSUM"))

        DVE_EXP_KC = (7,)

        def head_scores(h):
            ti, off = h // 2, 64 * (h % 2)
            prow = slice(off, off + CH)
            wexp = []
            for kcp in range(4):
                wt = wexp_pool.tile([P, 2, S], FP8, tag="wexp")
                for j in range(2):
                    kc = 2 * kcp + j
                    ps_s = spool.tile([P, S], F32, tag="ps")
                    for half in range(2):
                        sl = slice(half * 512, (half + 1) * 512)
                        nc.tensor.matmul(
                            ps_s[:, sl],
                            lhsT=k8[ti][prow, :, kc * P:(kc + 1) * P],
                            rhs=q8[ti][prow, :, sl],
                            start=True, stop=True, perf_mode=DR,
                        )
                    if kc in DVE_EXP_KC:
                        # exp(t) ~ 1 + t(1 + t/2), |t| < 0.5 (err < 1e-3)
                        tq = work.tile([P, S], BF16, tag="mod_tm")
                        nc.vector.tensor_scalar(
                            out=tq, in0=ps_s,
                            scalar1=1.0 / (64.0 * WS * WS), scalar2=0.0,
                            op0=OP.mult, op1=OP.bypass)
                        uq = work.tile([P, S], BF16, tag="res_tmp")
                        nc.vector.tensor_scalar(
                            out=uq, in0=tq, scalar1=0.5, scalar2=1.0,
                            op0=OP.mult, op1=OP.add)
                        nc.vector.tensor_tensor(uq, tq, uq, OP.mult)
                        nc.vector.tensor_scalar(
                            out=wt[:, j, :], in0=uq, scalar1=1.0, scalar2=1.0,
                            op0=OP.mult, op1=OP.add)
                    else:
                        nc.scalar.activation(wt[:, j, :], ps_s, AF.Exp,
                                             scale=1.0 / (64.0 * WS * WS))
                wexp.append(wt)
            return wexp

        def head_av(h, wexp):
            """AV matmul with the softmax denominator fused in: the
            stationary operand is [v_head | 32 ones-cols valued 2.0], so
            output rows 0:64 are y_unnorm and rows 64:96 are 2*den — one
            DoubleRow group at base partition 0 (ISA-safe), no extra cost
            (matmul cost is output free size only)."""
            ps_y = avpool.tile([P, S], F32, tag="ps_y")
            for half in range(2):
                sl = slice(half * 512, (half + 1) * 512)
                for kcp in range(4):
                    nc.tensor.matmul(
                        ps_y[0:CH + 32, sl],
                        lhsT=v2[kcp][:, :, h // 2, h % 2, :],
                        rhs=wexp[kcp][:, :, sl],
                        start=(kcp == 0), stop=(kcp == 3), perf_mode=DR,
                    )
            drow = att_tmp.tile([1, S], BF16, tag="drow", bufs=2)
            with nc.allow_low_precision(reason="softmax 1/den in bf16"):
                nc.vector.reciprocal(drow, ps_y[CH:CH + 1, :])  # 0.5/den
            rdb = rdb_pool.tile([P, S], BF16, tag="rdb")
            nc.gpsimd.partition_broadcast(rdb, drow)
            return ps_y, rdb

        def head_norm(h, ps_y, rdb):
            ti, j, off = h // 4, (h % 4) // 2, 64 * (h % 2)
            nc.vector.tensor_tensor(
                y8[ti][off:off + CH, j, :],
                ps_y[0:CH, :], rdb[0:CH, :], OP.mult,
            )

        def late_streams(step):
            if step == 2:
                nc.gpsimd.dma_start(wproj, t["w_proj8"])
            elif 4 <= step < 12:  # w_ada blocks 4..11
                wada_tiles[step] = wada_dma(step)
            elif 12 <= step < 20:  # w_mlp1 hi, 1MB pieces
                i = step - 12
                nc.gpsimd.dma_start(wmlp1[:, i // 2, :, (i % 2) * 2048:
                                          (i % 2) * 2048 + 2048],
                                    t["w_mlp18"][:, i // 2, :, (i % 2) * 2048:
                                                 (i % 2) * 2048 + 2048])
            elif 20 <= step < 28:  # w_mlp1 lo
                i = step - 20
                nc.gpsimd.dma_start(wmlp1l[:, i // 2, :, (i % 2) * 2048:
                                           (i % 2) * 2048 + 2048],
                                    t["w_mlp18l"][:, i // 2, :, (i % 2) * 2048:
                                                  (i % 2) * 2048 + 2048])

            if 6 <= step < 14:  # adaLN tail rides the scores psum ring
                hb = step - 2
                psx = spool.tile([P, S], F32, tag="ps")
                ada_halfblock(hb, wada_tiles.pop(hb), psx, hb * 4)

        st = {}
        for step in range(28):
            late_streams(step)
            if step < NH:
                st[step] = {"wexp": head_scores(step)}
            if 1 <= step and step - 1 < NH:
                hh = step - 1
                ps_y, rdb = head_av(hh, st[hh].pop("wexp"))
                st[hh]["ps_y"], st[hh]["rdb"] = ps_y, rdb
            if 2 <= step and step - 2 < NH:
                hh = step - 2
                head_norm(hh, st[hh].pop("ps_y"), st[hh].pop("rdb"))
                del st[hh]
        att_ps.close()


        nc.scalar.add(sc1[:, 8:16], cmod[:, 32:40], 1.0)
        gpr = const.tile([P, KH], F32, tag="gpr")
        nc.vector.tensor_scalar(out=gpr, in0=cmod[:, 16:24],
                                scalar1=1.0 / 512.0, scalar2=0.0,
                                op0=OP.mult, op1=OP.bypass)
        gpb = const.tile([P, KH], F32, tag="gpb")
        nc.vector.tensor_tensor(gpb, cmod[:, 16:24], b_proj_sb, OP.mult)
        gmr = const.tile([P, KH], F32, tag="gmr")
        nc.vector.tensor_scalar(out=gmr, in0=cmod[:, 40:48],
                                scalar1=1.0 / 32.0, scalar2=0.0,
                                op0=OP.mult, op1=OP.bypass)
        gmb = const.tile([P, KH], F32, tag="gmb")
        nc.vector.tensor_tensor(gmb, cmod[:, 40:48], b_mlp2_sb, OP.mult)

        # ---------------- proj + gated residual + LN2 stats -----------------
        ph3 = ExitStack()
        ps_pr = ph3.enter_context(tc.tile_pool(name="ps_pr", bufs=2, space="PSUM"))
        ps_ln2 = ph3.enter_context(tc.tile_pool(name="ps_ln2", bufs=1, space="PSUM"))
        pss2 = ps_ln2.tile([P, KS * KH], F32, tag="ln_s")
        psq2 = ps_ln2.tile([P, KS * KH], F32, tag="ln_q")
        for mc in range(KH):
            ps = ps_pr.tile([P, S], F32, tag="mm")
            for half in range(2):
                sl = slice(half * 512, (half + 1) * 512)
                for ti in range(4):
                    nc.tensor.matmul(
                        ps[:, sl],
                        lhsT=wproj[:, ti, :, mc * P:(mc + 1) * P],
                        rhs=y8[ti][:, :, sl],
                        start=(ti == 0), stop=(ti == 3), perf_mode=DR,
                    )
            tp = work.tile([P, S], BF16, tag="res_tmp")
            nc.scalar.activation(tp, ps, AF.Identity,
                                 bias=gpb[:, mc:mc + 1],
                                 scale=gpr[:, mc:mc + 1])
            nc.vector.tensor_tensor(xall[:, mc, :], xall[:, mc, :], tp, OP.add)
            # LN2 statistics for this chunk right away
            xsq = work.tile([P, S], BF16, tag="xsq")
            nc.vector.tensor_tensor(xsq, xall[:, mc, :], xall[:, mc, :],
                                    OP.mult)
            for tcv in range(KS):
                sl = slice(tcv * P, (tcv + 1) * P)
                col = tcv * KH + mc
                nc.tensor.matmul(
                    pss2[:, col:col + 1], lhsT=xall[:, mc, sl], rhs=ones_mv,
                    start=True, stop=True,
                )
                nc.tensor.matmul(
                    psq2[:, col:col + 1], lhsT=xsq[:, sl], rhs=ones_mv,
                    start=True, stop=True,
                )
        att_ctx.close()

        # ---------------- LN2 finish + modulate z2 + MLP --------------------
        ph4 = ExitStack()
        r2_b, mr2_b = ln_finish(pss2, psq2, SCR2_LN + 2 * S)
        ph3.close()
        wada_ctx.close()

        h_pool = ph4.enter_context(tc.tile_pool(name="h8", bufs=1))
        h8 = [h_pool.tile([P, 2, S], FP8, tag=f"h8_{i}", name=f"h8_{i}")
              for i in range(16)]
        z2_pool = ph4.enter_context(tc.tile_pool(name="z2", bufs=1))
        z2 = [z2_pool.tile([P, 2, S], FP8, tag=f"z2_{i}", name=f"z2_{i}")
              for i in range(4)]
        z2l = [z2_pool.tile([P, 2, S], FP8, tag=f"z2l_{i}", name=f"z2l_{i}")
               for i in range(4)]
        for half in range(2):
            sl = slice(half * 512, (half + 1) * 512)
            for kc in range(KH):
                # z_bf (bf16) -> z_hi (fp8) -> z_lo = fp8(z_bf - z_hi)
                tm = work.tile([P, S], BF16, tag="mod_tm")
                nc.vector.tensor_tensor(tm[:, sl], xall[:, kc, sl],
                                        r2_b[:, sl], OP.mult)
                nc.vector.tensor_tensor(tm[:, sl], tm[:, sl], mr2_b[:, sl],
                                        OP.subtract)
                zbf = work.tile([P, S], BF16, tag="stage_bf")
                nc.vector.tensor_scalar(
                    out=zbf[:, sl], in0=tm[:, sl],
                    scalar1=sc1[:, 8 + kc:8 + kc + 1],
                    scalar2=cmod[:, 24 + kc:24 + kc + 1],
                    op0=OP.mult, op1=OP.add,
                )
                zhi = z2[kc // 2][:, kc % 2, :]
                nc.scalar.copy(zhi[:, sl], zbf[:, sl])
                nc.gpsimd.tensor_tensor(z2l[kc // 2][:, kc % 2, sl],
                                        zbf[:, sl], zhi[:, sl], OP.subtract)

        wmlp2_pool = ph4.enter_context(tc.tile_pool(name="wmlp2", bufs=2))
        otmp_pool = ph4.enter_context(tc.tile_pool(name="otmp", bufs=2))

        def w2_blk_dma(mc):  # 1 out-chunk of hi+lo
            bh = wmlp2_pool.tile([P, 16, 2, P], FP8, tag="w2hi")
            nc.sync.dma_start(bh, t["w_mlp28"][:, :, :, mc * P:(mc + 1) * P])
            bl = wmlp2_pool.tile([P, 16, 2, P], FP8, tag="w2lo")
            nc.sync.dma_start(bl, t["w_mlp28l"][:, :, :, mc * P:(mc + 1) * P])
            return bh, bl

        w2blk = {0: w2_blk_dma(0), 1: w2_blk_dma(1)}

        m1_ctx = ExitStack()
        ps_m1 = m1_ctx.enter_context(tc.tile_pool(name="ps_m1", bufs=3, space="PSUM"))

        for mc in range(32):
            ps = ps_m1.tile([P, S], F32, tag="mm")
            for half in range(2):
                sl = slice(half * 512, (half + 1) * 512)
                for p_ in range(3):
                    wsrc = wmlp1 if p_ != 1 else wmlp1l
                    zsrc = z2 if p_ != 2 else z2l
                    for kcp in range(4):
                        nc.tensor.matmul(
                            ps[:, sl],
                            lhsT=wsrc[:, kcp, :, mc * P:(mc + 1) * P],
                            rhs=zsrc[kcp][:, :, sl],
                            start=(p_ == 0 and kcp == 0),
                            stop=(p_ == 2 and kcp == 3), perf_mode=DR,
                        )
            nc.scalar.activation(
                h8[mc // 2][:, mc % 2, :], ps, AF.Gelu_apprx_tanh,
                bias=b_mlp1_sb[:, mc:mc + 1], scale=1.0 / WS,
            )
        m1_ctx.close()

        w2_ctx = ExitStack()
        ps_m2 = w2_ctx.enter_context(tc.tile_pool(name="ps_m2", bufs=3, space="PSUM"))
        for mc in range(KH):
            if mc + 2 < KH:
                w2blk[mc + 2] = w2_blk_dma(mc + 2)
            bh, bl = w2blk.pop(mc)
            ps = ps_m2.tile([P, S], F32, tag="mm")
            off = 0
            for half in range(2):
                sl = slice(half * 512, (half + 1) * 512)
                for p_ in range(2):
                    wsrc = bh if p_ != 1 else bl
                    for kcp in range(16):
                        nc.tensor.matmul(
                            ps[:, sl],
                            lhsT=wsrc[:, kcp, :, off:off + P],
                            rhs=h8[kcp][:, :, sl],
                            start=(p_ == 0 and kcp == 0),
                            stop=(p_ == 1 and kcp == 15), perf_mode=DR,
                        )
            tp = work.tile([P, S], BF16, tag="res_tmp")
            nc.scalar.activation(tp, ps, AF.Identity,
                                 bias=gmb[:, mc:mc + 1],
                                 scale=gmr[:, mc:mc + 1])
            ot = otmp_pool.tile([P, S], F32, tag="ot")
            nc.vector.tensor_tensor(ot, xall[:, mc, :], tp, OP.add)
            nc.sync.dma_start(t["outT8"][:, mc, :], ot)
        w2_ctx.close()
        ph4.close()


@functools.lru_cache(maxsize=1)
def _get_nc():
    return _build_program()


def _fp8(a):
    return np.ascontiguousarray(
        np.clip(np.asarray(a, dtype=np.float32), -240.0, 240.0)
        .astype(ml_dtypes.float8_e4m3))


def kernel(x, c, w_ada, b_ada, w_qkv, w_proj, b_proj, w_mlp1, b_mlp1,
           w_mlp2, b_mlp2):
    nc = _get_nc()
    bf = ml_dtypes.bfloat16
    f32 = np.float32

    p = np.arange(128)
    w_qkv8 = _fp8((np.asarray(w_qkv, f32) * WS)
                  .reshape(4, 2, 128, 3 * H).transpose(2, 0, 1, 3))

    # --- w_proj row permutation matching the y8 layout ---
    phi = np.empty((128, 4, 2), np.int64)
    for ti in range(4):
        for j in range(2):
            phi[:, ti, j] = (4 * ti + 2 * j + p // 64) * CH + (p % 64)
    w_proj8 = _fp8((np.asarray(w_proj, f32) * WS)[phi])

    w1s = (np.asarray(w_mlp1, f32) * WS).reshape(4, 2, 128, 4 * H)\
        .transpose(2, 0, 1, 3)
    w_mlp18 = _fp8(w1s)
    w_mlp18l = _fp8(w1s - w_mlp18.astype(f32))
    w2s = (np.asarray(w_mlp2, f32) * WS).reshape(16, 2, 128, H)\
        .transpose(2, 0, 1, 3)
    w_mlp28 = _fp8(w2s)
    w_mlp28l = _fp8(w2s - w_mlp28.astype(f32))
    w_ada_t = np.ascontiguousarray(
        np.asarray(w_ada, f32).reshape(8, 128, 6, 1024)
        .transpose(1, 2, 0, 3).astype(bf))

    shared = {
        "w_ada_t": w_ada_t,
        "b_ada": np.ascontiguousarray(b_ada, dtype=f32),
        "w_qkv8": w_qkv8,
        "w_proj8": w_proj8,
        "b_proj": np.ascontiguousarray(b_proj, dtype=f32),
        "w_mlp18": w_mlp18,
        "w_mlp18l": w_mlp18l,
        "b_mlp1": np.ascontiguousarray(b_mlp1, dtype=f32),
        "w_mlp28": w_mlp28,
        "w_mlp28l": w_mlp28l,
        "b_mlp2": np.ascontiguousarray(b_mlp2, dtype=f32),
    }
    in_maps = []
    for bidx in range(N_CORES):
        m = dict(shared)
        m["xT8"] = np.ascontiguousarray(
            np.asarray(x[bidx], f32).T.reshape(8, 128, S)
            .transpose(1, 0, 2).astype(bf))
        m["cvec"] = np.ascontiguousarray(np.asarray(c[bidx], dtype=f32))
        in_maps.append(m)

    res = run_bass_kernel_spmd(
        nc, in_maps, core_ids=list(range(N_CORES)), trace=False
    )
    kernel.last_results = res

    out = np.empty((B, S, H), dtype=f32)
    for bidx in range(N_CORES):
        o = np.asarray(res.results[bidx]["outT8"])  # [128, 8, S]
        out[bidx] = o.transpose(1, 0, 2).reshape(H, S).T
    return out


if __name__ == "__main__":
    nc = _get_nc()
    print("program built ok")



# revision 51
# speedup vs baseline: 1.0883x; 1.0030x over previous
"""DiT block kernel for Trainium2, data-parallel over batch (8 cores, B=8).

v2: fp8 DoubleRow matmuls for qkv/scores/AV/proj/mlp1/mlp2 (2 K-chunks per
pass), LN statistics and adaLN GEMV computed with x/w_ada as the *stationary*
operand and a tiny moving operand (cost ~ output free size), per-token
quantities (LN rstd, softmax 1/den) produced directly in token-on-partition
layout via PE transposes of single rows, elementwise work spread across
DVE / ACT / GPSIMD.

Layouts (host-retiled):
  xT8      [128, 8, 1024]  bf16  x[b].T tiled: [p, kc, s] = x[b, s, kc*128+p]
  w_qkv8   [128, 4, 2, 3072] fp8 (x32), k/q column blocks permuted so that
           head h lives on partitions 32*(h%4).. with c split across the
           DoubleRow slot dim (c%32 on partitions, c//32 on slot)
  w_proj8  [128, 4, 2, 1024] fp8 (x32), rows permuted to match the y8 layout
           head h -> tile h//4, slot (h%4)//2, partition base 64*(h%2)
  w_mlp18  [128, 4, 2, 4096] fp8 (x32)
  w_mlp28  [128, 16, 2, 1024] fp8 (x32)
  w_ada_t  [128, 6, 8, 1024] bf16  [p, blk, kc, m] = w_ada[kc*128+p, blk*1024+m]
  outT8    [128, 8, 1024]  f32   [p, mc, s] = out[b, s, mc*128+p]

Scale bookkeeping: weights x32 in fp8. scoresT psum = 1024*k.q -> exp scale
1/(64*1024). AV y8 = 16*y_true (rd = 0.5/den). proj psum = 512*attn -> gate/512.
mlp1 psum = 32*pre -> gelu scale 1/32. mlp2 psum = 32*mlp -> gate/32.
"""

import os
import sys
import functools
from contextlib import ExitStack

import numpy as np

for _p in ("/opt/trn_rl_repo", "/root/.axon_site/_ro/trn_rl_repo"):
    if os.path.isdir(_p) and _p not in sys.path:
        sys.path.insert(0, _p)

import ml_dtypes  # noqa: E402
import concourse.bass as bass  # noqa: E402
from concourse import bacc  # noqa: E402
import concourse.tile as tile  # noqa: E402
from concourse import mybir  # noqa: E402
from concourse.bass_utils import run_bass_kernel_spmd  # noqa: E402

F32 = mybir.dt.float32
BF16 = mybir.dt.bfloat16
FP8 = mybir.dt.float8e4
AF = mybir.ActivationFunctionType
OP = mybir.AluOpType
DR = mybir.MatmulPerfMode.DoubleRow

B, S, H, NH, CH = 8, 1024, 1024, 16, 64
P = 128
KH = H // P          # 8 chunks over H
KS = S // P          # 8 chunks over S
EPS = 1e-6
N_CORES = 8
WS = 32.0            # fp8 weight scale

# scr2 (bf16) scratch layout: LN r/mr (2 LNs x 2 rows x S) then per-head rd
SCR2_LN = 0
SCR2_HEAD = 4 * S
SCR2_N = SCR2_HEAD + NH * S


def _build_program():
    nc = bacc.Bacc("TRN2", target_bir_lowering=False, debug=False)

    t = {}
    t["xT8"] = nc.dram_tensor("xT8", (P, KH, S), BF16, kind="ExternalInput").ap()
    t["cvec"] = nc.dram_tensor("cvec", (H,), F32, kind="ExternalInput").ap()
    t["w_ada_t"] = nc.dram_tensor("w_ada_t", (P, 6, KH, 1024), BF16,
                                  kind="ExternalInput").ap()
    t["b_ada"] = nc.dram_tensor("b_ada", (6 * H,), F32, kind="ExternalInput").ap()
    t["w_qkv8"] = nc.dram_tensor("w_qkv8", (P, 4, 2, 3 * H), FP8,
                                 kind="ExternalInput").ap()
    t["w_proj8"] = nc.dram_tensor("w_proj8", (P, 4, 2, H), FP8,
                                  kind="ExternalInput").ap()
    t["b_proj"] = nc.dram_tensor("b_proj", (H,), F32, kind="ExternalInput").ap()
    t["w_mlp18"] = nc.dram_tensor("w_mlp18", (P, 4, 2, 4 * H), FP8,
                                  kind="ExternalInput").ap()
    t["w_mlp18l"] = nc.dram_tensor("w_mlp18l", (P, 4, 2, 4 * H), FP8,
                                   kind="ExternalInput").ap()
    t["b_mlp1"] = nc.dram_tensor("b_mlp1", (4 * H,), F32, kind="ExternalInput").ap()
    t["w_mlp28"] = nc.dram_tensor("w_mlp28", (P, 16, 2, H), FP8,
                                  kind="ExternalInput").ap()
    t["w_mlp28l"] = nc.dram_tensor("w_mlp28l", (P, 16, 2, H), FP8,
                                   kind="ExternalInput").ap()
    t["b_mlp2"] = nc.dram_tensor("b_mlp2", (H,), F32, kind="ExternalInput").ap()
    t["outT8"] = nc.dram_tensor("outT8", (P, KH, S), F32, kind="ExternalOutput").ap()
    t["scr2"] = nc.dram_tensor("scr2", (SCR2_N,), BF16, kind="ExternalOutput").ap()

    with tile.TileContext(nc) as tc:
        _emit(tc, t)
    nc.compile()
    return nc


def _emit(tc, t):
    nc = tc.nc
    scr2 = t["scr2"]

    def pbcast(ap_1p, nparts):
        """Partition-broadcast view of a 1-partition (DRAM) AP."""
        return bass.AP(
            tensor=ap_1p.tensor, offset=ap_1p.offset,
            ap=[[0, nparts]] + list(ap_1p.ap[1:]),
        )

    def scr2_row(off, n):
        return scr2[off:off + n].rearrange("(a n) -> a n", a=1)

    def scr2_tok(off, n):
        """[128, n//128] view; flat[k*128+p] = element [p, k]."""
        return scr2[off:off + n].rearrange("(k p) -> p k", p=P)

    with ExitStack() as ctx:
        const = ctx.enter_context(tc.tile_pool(name="const", bufs=1))
        rows = ctx.enter_context(tc.tile_pool(name="rows", bufs=1))
        work = ctx.enter_context(tc.tile_pool(name="work", bufs=2))
        xpool = ctx.enter_context(tc.tile_pool(name="xpool", bufs=1))
        bcast = ctx.enter_context(tc.tile_pool(name="bcast", bufs=1))
        wmlp1_pool = ctx.enter_context(tc.tile_pool(name="wmlp1", bufs=1))
        wmlp1 = wmlp1_pool.tile([P, 4, 2, 4 * H], FP8, tag="wmlp1")
        wmlp1l = wmlp1_pool.tile([P, 4, 2, 4 * H], FP8, tag="wmlp1l")
        wada_ctx = ExitStack()
        wada_pool = wada_ctx.enter_context(tc.tile_pool(name="wada", bufs=1))

        # ---------------- constants ----------------------------------------
        ones_mv = const.tile([P, 1], BF16, tag="ones_mv")
        nc.vector.memset(ones_mv, 1.0)

        c_sb = const.tile([P, KH], F32, tag="c_sb")
        nc.gpsimd.dma_start(c_sb, t["cvec"].rearrange("(k p) -> p k", p=P))
        b_ada_sb = const.tile([P, 48], F32, tag="b_ada_sb")
        nc.gpsimd.dma_start(b_ada_sb, t["b_ada"].rearrange("(k p) -> p k", p=P))
        b_proj_sb = const.tile([P, KH], F32, tag="b_proj_sb")
        nc.gpsimd.dma_start(b_proj_sb, t["b_proj"].rearrange("(k p) -> p k", p=P))
        b_mlp1_sb = const.tile([P, 32], F32, tag="b_mlp1_sb")
        nc.gpsimd.dma_start(b_mlp1_sb, t["b_mlp1"].rearrange("(k p) -> p k", p=P))
        b_mlp2_sb = const.tile([P, KH], F32, tag="b_mlp2_sb")
        nc.gpsimd.dma_start(b_mlp2_sb, t["b_mlp2"].rearrange("(k p) -> p k", p=P))

        # ---------------- input x + first w_ada half-blocks ----------------
        xall = xpool.tile([P, KH, S], BF16, tag="xall")
        nc.sync.dma_start(xall[:, 0:4, :], t["xT8"][:, 0:4, :])
        nc.sync.dma_start(xall[:, 4:8, :], t["xT8"][:, 4:8, :])

        # w_ada streamed as 12 half-blocks [P, KH, 512] (4 cmod cols each)
        def wada_dma(hb, eng=None):
            wt = wada_pool.tile([P, KH, 512], BF16, tag="wada")
            blk, mlo = hb // 2, (hb % 2) * 512
            (eng or nc.gpsimd).dma_start(
                wt, t["w_ada_t"][:, blk, :, mlo:mlo + 512])
            return wt

        wada_tiles = {hb: wada_dma(hb, nc.sync) for hb in range(2)}

        # ---------------- silu(c) ------------------------------------------
        sc_sb = const.tile([P, KH], BF16, tag="sc_sb")
        nc.scalar.activation(sc_sb, c_sb, AF.Silu)

        # attention SBUF tiles (allocated early for pool stack order)
        att_ctx = ExitStack()
        kq_pool = att_ctx.enter_context(tc.tile_pool(name="kq", bufs=1))
        k8 = [kq_pool.tile([P, 2, S], FP8, tag=f"k8_{i}", name=f"k8_{i}")
              for i in range(KS)]
        q8 = [kq_pool.tile([P, 2, S], FP8, tag=f"q8_{i}", name=f"q8_{i}")
              for i in range(KS)]
        for i in range(KS):
            nc.vector.memset(k8[i][:, 1, :].bitcast(mybir.dt.uint32), 0)
            nc.vector.memset(q8[i][:, 1, :].bitcast(mybir.dt.uint32), 0)
        v2 = [kq_pool.tile([P, 2, NH // 2, 2, CH + 32], FP8, tag=f"v2_{i}",
                           name=f"v2_{i}")
              for i in range(4)]
        for i in range(4):
            nc.vector.memset(v2[i][:, :, :, :, CH:CH + 32], 2.0)
        y8 = [kq_pool.tile([P, 2, S], FP8, tag=f"y8_{i}", name=f"y8_{i}")
              for i in range(4)]

        # ---------------- phase-1 psum pools --------------------------------
        ph1 = ExitStack()
        ps_ada = ph1.enter_context(tc.tile_pool(name="ps_ada", bufs=1, space="PSUM"))
        psada = ps_ada.tile([P, 48], F32, tag="ada")
        cmod = const.tile([P, 48], F32, tag="cmod")

        ln1_ps = ExitStack()
        ps_ln = ln1_ps.enter_context(tc.tile_pool(name="ps_ln", bufs=1, space="PSUM"))

        # ---------------- LN statistics (x stationary, ones moving) --------
        def ln_stats(pool, src, xsq_tag):
            """Returns psum [128, KS*KH] partial sums & sumsq (col tcv*KH+kc),
            token s = tc*128 + p. Each matmul is an independent start/stop
            group: interleaved accumulation in one PSUM bank is NOT safe (the
            start flag marks the whole 2 KiB bank pending-zero, wiping other
            columns' later accumulating writes), but completed columns' data
            survives subsequent starts."""
            pss = pool.tile([P, KS * KH], F32, tag="ln_s")
            psq = pool.tile([P, KS * KH], F32, tag="ln_q")
            for kc in range(KH):
                xsq = work.tile([P, S], BF16, tag=xsq_tag)
                nc.scalar.activation(xsq, src[:, kc, :], AF.Square)
                for tcv in range(KS):
                    sl = slice(tcv * P, (tcv + 1) * P)
                    col = tcv * KH + kc
                    nc.tensor.matmul(
                        pss[:, col:col + 1], lhsT=src[:, kc, sl], rhs=ones_mv,
                        start=True, stop=True,
                    )
                    nc.tensor.matmul(
                        psq[:, col:col + 1], lhsT=xsq[:, sl], rhs=ones_mv,
                        start=True, stop=True,
                    )
            return pss, psq

        def ln_finish(pss, psq, o2_base):
            """rstd & mean*rstd from [128, KS] stats; bf16 via scr2 to
            partition-broadcast tiles [128, S]."""
            pssum = rows.tile([P, KS], F32, tag="pssum")
            nc.vector.tensor_reduce(
                pssum, pss.rearrange("p (t k) -> p t k", t=KS),
                axis=mybir.AxisListType.X, op=OP.add)
            psqs = rows.tile([P, KS], F32, tag="psqs")
            nc.vector.tensor_reduce(
                psqs, psq.rearrange("p (t k) -> p t k", t=KS),
                axis=mybir.AxisListType.X, op=OP.add)
            m = rows.tile([P, KS], F32, tag="m_tok")
            nc.vector.tensor_scalar(out=m, in0=pssum, scalar1=1.0 / H,
                                    scalar2=0.0, op0=OP.mult, op1=OP.bypass)
            v = rows.tile([P, KS], F32, tag="v_tok")
            nc.vector.tensor_scalar(out=v, in0=psqs, scalar1=1.0 / H,
                                    scalar2=EPS, op0=OP.mult, op1=OP.add)
            msq = rows.tile([P, KS], F32, tag="msq_tok")
            nc.vector.tensor_tensor(msq, m, m, OP.mult)
            nc.vector.tensor_tensor(v, v, msq, OP.subtract)
            r = rows.tile([P, KS], F32, tag="r_tok")
            nc.vector.tensor_scalar(out=r, in0=v, scalar1=-0.5, scalar2=1.5,
                                    op0=OP.mult, op1=OP.add)
            s = rows.tile([P, KS], F32, tag="s_tok")
            for _ in range(2):
                nc.vector.tensor_tensor(s, r, r, OP.mult)
                nc.vector.tensor_tensor(s, s, v, OP.mult)
                nc.vector.tensor_scalar(out=s, in0=s, scalar1=-0.5, scalar2=1.5,
                                        op0=OP.mult, op1=OP.add)
                nc.vector.tensor_tensor(r, r, s, OP.mult)
            nc.vector.tensor_tensor(m, m, r, OP.mult)  # m <- m * r
            rb16 = rows.tile([P, KS], BF16, tag="rb16")
            nc.vector.tensor_copy(rb16, r)
            mb16 = rows.tile([P, KS], BF16, tag="mb16")
            nc.vector.tensor_copy(mb16, m)
            nc.gpsimd.dma_start(scr2_tok(o2_base, S), rb16)
            nc.gpsimd.dma_start(scr2_tok(o2_base + S, S), mb16)
            rrow = rows.tile([1, S], BF16, tag="rrow")
            mrow = rows.tile([1, S], BF16, tag="mrow")
            nc.sync.dma_start(rrow, scr2_row(o2_base, S))
            nc.sync.dma_start(mrow, scr2_row(o2_base + S, S))
            r_b = bcast.tile([P, S], BF16, tag="r_b")
            mr_b = bcast.tile([P, S], BF16, tag="mr_b")
            nc.gpsimd.partition_broadcast(r_b, rrow)
            nc.gpsimd.partition_broadcast(mr_b, mrow)
            return r_b, mr_b

        pss1, psq1 = ln_stats(ps_ln, xall, "xsq")

        # adaLN GEMV: one half-block = 4 cmod columns of 128
        def ada_halfblock(hb, wt, ps, base):
            for mcol in range(4):
                col = hb * 4 + mcol - base
                for kc in range(KH):
                    nc.tensor.matmul(
                        ps[:, col:col + 1],
                        lhsT=wt[:, kc, mcol * P:(mcol + 1) * P],
                        rhs=sc_sb[:, kc:kc + 1],
                        start=(kc == 0), stop=(kc == KH - 1),
                    )
            nc.vector.tensor_tensor(
                cmod[:, hb * 4:(hb + 1) * 4],
                ps[:, hb * 4 - base:(hb + 1) * 4 - base],
                b_ada_sb[:, hb * 4:(hb + 1) * 4], OP.add,
            )

        for hb in range(2):  # shift_msa
            ada_halfblock(hb, wada_tiles.pop(hb), psada, 0)

        r1_b, mr1_b = ln_finish(pss1, psq1, SCR2_LN)
        for hb in range(2, 4):  # scale_msa, behind the r/mr loads in the FIFO
            ada_halfblock(hb, wada_dma(hb), psada, 0)
        ln1_ps.close()

        sc1 = const.tile([P, 16], F32, tag="sc1")  # 1+scale_msa | 1+scale_mlp
        nc.scalar.add(sc1[:, 0:8], cmod[:, 8:16], 1.0)

        # ---------------- z1 modulate + qkv (DoubleRow fp8) ----------------
        zpool = ExitStack()
        z1_pool = zpool.enter_context(tc.tile_pool(name="z1", bufs=1))
        z1 = [z1_pool.tile([P, 2, S], FP8, tag=f"z1_{i}", name=f"z1_{i}")
              for i in range(4)]

        wqkv_ctx = ExitStack()
        wqkv_pool = wqkv_ctx.enter_context(tc.tile_pool(name="wqkv", bufs=1))
        wqkv = wqkv_pool.tile([P, 4, 2, 3 * H], FP8, tag="wqkv")
        for sec in range(3):
            nc.gpsimd.dma_start(
                wqkv[:, :, :, sec * H:(sec + 1) * H],
                t["w_qkv8"][:, :, :, sec * H:(sec + 1) * H],
            )

        def modulate(dst, src, r_b, mr_b, col, shift_ap, sl):
            tm = work.tile([P, S], BF16, tag="mod_tm")
            nc.vector.tensor_tensor(tm[:, sl], src[:, sl], r_b[:, sl], OP.mult)
            nc.vector.tensor_tensor(tm[:, sl], tm[:, sl], mr_b[:, sl],
                                    OP.subtract)
            nc.scalar.activation(dst[:, sl], tm[:, sl], AF.Identity,
                                 bias=shift_ap, scale=sc1[:, col:col + 1])

        for half in range(2):
            sl = slice(half * 512, (half + 1) * 512)
            for kc in range(KH):
                modulate(z1[kc // 2][:, kc % 2, :], xall[:, kc, :], r1_b, mr1_b,
                         kc, cmod[:, kc:kc + 1], sl)

        ps_mm_ctx = ExitStack()
        ps_mm = ps_mm_ctx.enter_context(
            tc.tile_pool(name="ps_mm", bufs=3, space="PSUM"))

        for oc in range(16):  # 8 k-chunks then 8 q-chunks
            ps = ps_mm.tile([P, S], F32, tag="mm")
            for half in range(2):
                sl = slice(half * 512, (half + 1) * 512)
                for kcp in range(4):
                    nc.tensor.matmul(
                        ps[:, sl],
                        lhsT=wqkv[:, kcp, :, oc * P:(oc + 1) * P],
                        rhs=z1[kcp][:, :, sl],
                        start=(kcp == 0), stop=(kcp == 3), perf_mode=DR,
                    )
            dst = k8 if oc < 8 else q8
            nc.scalar.copy(dst[oc % 8][:, 0, :], ps)

        for sc in range(KS):  # v, token-major
            ps = ps_mm.tile([P, S], F32, tag="mm")
            for half in range(2):
                sl = slice(2048 + half * 512, 2048 + (half + 1) * 512)
                osl = slice(half * 512, (half + 1) * 512)
                for kcp in range(4):
                    nc.tensor.matmul(
                        ps[:, osl],
                        lhsT=z1[kcp][:, :, sc * P:(sc + 1) * P],
                        rhs=wqkv[:, kcp, :, sl],
                        start=(kcp == 0), stop=(kcp == 3), perf_mode=DR,
                    )
            nc.scalar.copy(
                v2[sc // 2][:, sc % 2, :, :, 0:CH],
                ps.rearrange("p (hp two c) -> p hp two c", hp=NH // 2, two=2),
            )
        ps_mm_ctx.close()
        wqkv_ctx.close()
        zpool.close()
        ph1.close()

        # ---------------- attention ----------------------------------------
        wexp_pool = att_ctx.enter_context(tc.tile_pool(name="wexp", bufs=8))
        att_tmp = att_ctx.enter_context(tc.tile_pool(name="att_tmp", bufs=3))
        rdb_pool = att_ctx.enter_context(tc.tile_pool(name="rdb", bufs=3))
        wproj_pool = att_ctx.enter_context(tc.tile_pool(name="wproj", bufs=1))
        wproj = wproj_pool.tile([P, 4, 2, H], FP8, tag="wproj")

        att_ps = ExitStack()
        spool = att_ps.enter_context(tc.tile_pool(name="spool", bufs=2, space="PSUM"))
        avpool = att_ps.enter_context(tc.tile_pool(name="avpool", bufs=2, space="PSUM"))

        DVE_EXP_KC = ()

        def head_scores(h):
            ti, off = h // 2, 64 * (h % 2)
            prow = slice(off, off + CH)
            wexp = []
            for kcp in range(4):
                wt = wexp_pool.tile([P, 2, S], FP8, tag="wexp")
                for j in range(2):
                    kc = 2 * kcp + j
                    ps_s = spool.tile([P, S], F32, tag="ps")
                    for half in range(2):
                        sl = slice(half * 512, (half + 1) * 512)
                        nc.tensor.matmul(
                            ps_s[:, sl],
                            lhsT=k8[ti][prow, :, kc * P:(kc + 1) * P],
                            rhs=q8[ti][prow, :, sl],
                            start=True, stop=True, perf_mode=DR,
                        )
                    if kc in DVE_EXP_KC:
                        # exp(t) ~ 1 + t(1 + t/2), |t| < 0.5 (err < 1e-3)
                        tq = work.tile([P, S], BF16, tag="mod_tm")
                        nc.vector.tensor_scalar(
                            out=tq, in0=ps_s,
                            scalar1=1.0 / (64.0 * WS * WS), scalar2=0.0,
                            op0=OP.mult, op1=OP.bypass)
                        uq = work.tile([P, S], BF16, tag="res_tmp")
                        nc.vector.tensor_scalar(
                            out=uq, in0=tq, scalar1=0.5, scalar2=1.0,
                            op0=OP.mult, op1=OP.add)
                        nc.vector.tensor_tensor(uq, tq, uq, OP.mult)
                        nc.vector.tensor_scalar(
                            out=wt[:, j, :], in0=uq, scalar1=1.0, scalar2=1.0,
                            op0=OP.mult, op1=OP.add)
                    else:
                        nc.scalar.activation(wt[:, j, :], ps_s, AF.Exp,
                                             scale=1.0 / (64.0 * WS * WS))
                wexp.append(wt)
            return wexp

        def head_av(h, wexp):
            """AV matmul with the softmax denominator fused in: the
            stationary operand is [v_head | 32 ones-cols valued 2.0], so
            output rows 0:64 are y_unnorm and rows 64:96 are 2*den — one
            DoubleRow group at base partition 0 (ISA-safe), no extra cost
            (matmul cost is output free size only)."""
            ps_y = avpool.tile([P, S], F32, tag="ps_y")
            for half in range(2):
                sl = slice(half * 512, (half + 1) * 512)
                for kcp in range(4):
                    nc.tensor.matmul(
                        ps_y[0:CH + 32, sl],
                        lhsT=v2[kcp][:, :, h // 2, h % 2, :],
                        rhs=wexp[kcp][:, :, sl],
                        start=(kcp == 0), stop=(kcp == 3), perf_mode=DR,
                    )
            drow = att_tmp.tile([1, S], BF16, tag="drow", bufs=2)
            with nc.allow_low_precision(reason="softmax 1/den in bf16"):
                nc.vector.reciprocal(drow, ps_y[CH:CH + 1, :])  # 0.5/den
            rdb = rdb_pool.tile([P, S], BF16, tag="rdb")
            nc.gpsimd.partition_broadcast(rdb, drow)
            return ps_y, rdb

        def head_norm(h, ps_y, rdb):
            ti, j, off = h // 4, (h % 4) // 2, 64 * (h % 2)
            nc.vector.tensor_tensor(
                y8[ti][off:off + CH, j, :],
                ps_y[0:CH, :], rdb[0:CH, :], OP.mult,
            )

        def late_streams(step):
            if step == 2:
                nc.gpsimd.dma_start(wproj, t["w_proj8"])
            elif 4 <= step < 12:  # w_ada blocks 4..11
                wada_tiles[step] = wada_dma(step)
            elif 12 <= step < 20:  # w_mlp1 hi, 1MB pieces
                i = step - 12
                nc.gpsimd.dma_start(wmlp1[:, i // 2, :, (i % 2) * 2048:
                                          (i % 2) * 2048 + 2048],
                                    t["w_mlp18"][:, i // 2, :, (i % 2) * 2048:
                                                 (i % 2) * 2048 + 2048])
            elif 20 <= step < 28:  # w_mlp1 lo
                i = step - 20
                nc.gpsimd.dma_start(wmlp1l[:, i // 2, :, (i % 2) * 2048:
                                           (i % 2) * 2048 + 2048],
                                    t["w_mlp18l"][:, i // 2, :, (i % 2) * 2048:
                                                  (i % 2) * 2048 + 2048])

            if 6 <= step < 14:  # adaLN tail rides the scores psum ring
                hb = step - 2
                psx = spool.tile([P, S], F32, tag="ps")
                ada_halfblock(hb, wada_tiles.pop(hb), psx, hb * 4)

        st = {}
        for step in range(28):
            late_streams(step)
            if step < NH:
                st[step] = {"wexp": head_scores(step)}
            if 1 <= step and step - 1 < NH:
                hh = step - 1
                ps_y, rdb = head_av(hh, st[hh].pop("wexp"))
                st[hh]["ps_y"], st[hh]["rdb"] = ps_y, rdb
            if 2 <= step and step - 2 < NH:
                hh = step - 2
                head_norm(hh, st[hh].pop("ps_y"), st[hh].pop("rdb"))
                del st[hh]
        att_ps.close()


        nc.scalar.add(sc1[:, 8:16], cmod[:, 32:40], 1.0)
        gpr = const.tile([P, KH], F32, tag="gpr")
        nc.vector.tensor_scalar(out=gpr, in0=cmod[:, 16:24],
                                scalar1=1.0 / 512.0, scalar2=0.0,
                                op0=OP.mult, op1=OP.bypass)
        gpb = const.tile([P, KH], F32, tag="gpb")
        nc.vector.tensor_tensor(gpb, cmod[:, 16:24], b_proj_sb, OP.mult)
        gmr = const.tile([P, KH], F32, tag="gmr")
        nc.vector.tensor_scalar(out=gmr, in0=cmod[:, 40:48],
                                scalar1=1.0 / 32.0, scalar2=0.0,
                                op0=OP.mult, op1=OP.bypass)
        gmb = const.tile([P, KH], F32, tag="gmb")
        nc.vector.tensor_tensor(gmb, cmod[:, 40:48], b_mlp2_sb, OP.mult)

        # ---------------- proj + gated residual + LN2 stats -----------------
        ph3 = ExitStack()
        ps_pr = ph3.enter_context(tc.tile_pool(name="ps_pr", bufs=2, space="PSUM"))
        ps_ln2 = ph3.enter_context(tc.tile_pool(name="ps_ln2", bufs=1, space="PSUM"))
        pss2 = ps_ln2.tile([P, KS * KH], F32, tag="ln_s")
        psq2 = ps_ln2.tile([P, KS * KH], F32, tag="ln_q")
        for mc in range(KH):
            ps = ps_pr.tile([P, S], F32, tag="mm")
            for half in range(2):
                sl = slice(half * 512, (half + 1) * 512)
                for ti in range(4):
                    nc.tensor.matmul(
                        ps[:, sl],
                        lhsT=wproj[:, ti, :, mc * P:(mc + 1) * P],
                        rhs=y8[ti][:, :, sl],
                        start=(ti == 0), stop=(ti == 3), perf_mode=DR,
                    )
            tp = work.tile([P, S], BF16, tag="res_tmp")
            nc.scalar.activation(tp, ps, AF.Identity,
                                 bias=gpb[:, mc:mc + 1],
                                 scale=gpr[:, mc:mc + 1])
            nc.vector.tensor_tensor(xall[:, mc, :], xall[:, mc, :], tp, OP.add)
            # LN2 statistics for this chunk right away
            xsq = work.tile([P, S], BF16, tag="xsq")
            nc.vector.tensor_tensor(xsq, xall[:, mc, :], xall[:, mc, :],
                                    OP.mult)
            for tcv in range(KS):
                sl = slice(tcv * P, (tcv + 1) * P)
                col = tcv * KH + mc
                nc.tensor.matmul(
                    pss2[:, col:col + 1], lhsT=xall[:, mc, sl], rhs=ones_mv,
                    start=True, stop=True,
                )
                nc.tensor.matmul(
                    psq2[:, col:col + 1], lhsT=xsq[:, sl], rhs=ones_mv,
                    start=True, stop=True,
                )
        att_ctx.close()

        # ---------------- LN2 finish + modulate z2 + MLP --------------------
        ph4 = ExitStack()
        r2_b, mr2_b = ln_finish(pss2, psq2, SCR2_LN + 2 * S)
        ph3.close()
        wada_ctx.close()

        h_pool = ph4.enter_context(tc.tile_pool(name="h8", bufs=1))
        h8 = [h_pool.tile([P, 2, S], FP8, tag=f"h8_{i}", name=f"h8_{i}")
              for i in range(16)]
        z2_pool = ph4.enter_context(tc.tile_pool(name="z2", bufs=1))
        z2 = [z2_pool.tile([P, 2, S], FP8, tag=f"z2_{i}", name=f"z2_{i}")
              for i in range(4)]
        z2l = [z2_pool.tile([P, 2, S], FP8, tag=f"z2l_{i}", name=f"z2l_{i}")
               for i in range(4)]
        for half in range(2):
            sl = slice(half * 512, (half + 1) * 512)
            for kc in range(KH):
                # z_bf (bf16) -> z_hi (fp8) -> z_lo = fp8(z_bf - z_hi)
                tm = work.tile([P, S], BF16, tag="mod_tm")
                nc.vector.tensor_tensor(tm[:, sl], xall[:, kc, sl],
                                        r2_b[:, sl], OP.mult)
                nc.vector.tensor_tensor(tm[:, sl], tm[:, sl], mr2_b[:, sl],
                                        OP.subtract)
                zbf = work.tile([P, S], BF16, tag="stage_bf")
                nc.vector.tensor_scalar(
                    out=zbf[:, sl], in0=tm[:, sl],
                    scalar1=sc1[:, 8 + kc:8 + kc + 1],
                    scalar2=cmod[:, 24 + kc:24 + kc + 1],
                    op0=OP.mult, op1=OP.add,
                )
                zhi = z2[kc // 2][:, kc % 2, :]
                nc.scalar.copy(zhi[:, sl], zbf[:, sl])
                nc.gpsimd.tensor_tensor(z2l[kc // 2][:, kc % 2, sl],
                                        zbf[:, sl], zhi[:, sl], OP.subtract)

        wmlp2_pool = ph4.enter_context(tc.tile_pool(name="wmlp2", bufs=2))
        otmp_pool = ph4.enter_context(tc.tile_pool(name="otmp", bufs=2))

        def w2_blk_dma(mc):  # 1 out-chunk of hi+lo
            bh = wmlp2_pool.tile([P, 16, 2, P], FP8, tag="w2hi")
            nc.sync.dma_start(bh, t["w_mlp28"][:, :, :, mc * P:(mc + 1) * P])
            bl = wmlp2_pool.tile([P, 16, 2, P], FP8, tag="w2lo")
            nc.sync.dma_start(bl, t["w_mlp28l"][:, :, :, mc * P:(mc + 1) * P])
            return bh, bl

        w2blk = {0: w2_blk_dma(0), 1: w2_blk_dma(1)}

        m1_ctx = ExitStack()
        ps_m1 = m1_ctx.enter_context(tc.tile_pool(name="ps_m1", bufs=3, space="PSUM"))

        for mc in range(32):
            ps = ps_m1.tile([P, S], F32, tag="mm")
            for half in range(2):
                sl = slice(half * 512, (half + 1) * 512)
                for p_ in range(3):
                    wsrc = wmlp1 if p_ != 1 else wmlp1l
                    zsrc = z2 if p_ != 2 else z2l
                    for kcp in range(4):
                        nc.tensor.matmul(
                            ps[:, sl],
                            lhsT=wsrc[:, kcp, :, mc * P:(mc + 1) * P],
                            rhs=zsrc[kcp][:, :, sl],
                            start=(p_ == 0 and kcp == 0),
                            stop=(p_ == 2 and kcp == 3), perf_mode=DR,
                        )
            nc.scalar.activation(
                h8[mc // 2][:, mc % 2, :], ps, AF.Gelu_apprx_tanh,
                bias=b_mlp1_sb[:, mc:mc + 1], scale=1.0 / WS,
            )
        m1_ctx.close()

        w2_ctx = ExitStack()
        ps_m2 = w2_ctx.enter_context(tc.tile_pool(name="ps_m2", bufs=3, space="PSUM"))
        for mc in range(KH):
            if mc + 2 < KH:
                w2blk[mc + 2] = w2_blk_dma(mc + 2)
            bh, bl = w2blk.pop(mc)
            ps = ps_m2.tile([P, S], F32, tag="mm")
            off = 0
            for half in range(2):
                sl = slice(half * 512, (half + 1) * 512)
                for p_ in range(2):
                    wsrc = bh if p_ != 1 else bl
                    for kcp in range(16):
                        nc.tensor.matmul(
                            ps[:, sl],
                            lhsT=wsrc[:, kcp, :, off:off + P],
                            rhs=h8[kcp][:, :, sl],
                            start=(p_ == 0 and kcp == 0),
                            stop=(p_ == 1 and kcp == 15), perf_mode=DR,
                        )
            tp = work.tile([P, S], BF16, tag="res_tmp")
            nc.scalar.activation(tp, ps, AF.Identity,
                                 bias=gmb[:, mc:mc + 1],
                                 scale=gmr[:, mc:mc + 1])
            ot = otmp_pool.tile([P, S], F32, tag="ot")
            nc.vector.tensor_tensor(ot, xall[:, mc, :], tp, OP.add)
            nc.sync.dma_start(t["outT8"][:, mc, :], ot)
        w2_ctx.close()
        ph4.close()


@functools.lru_cache(maxsize=1)
def _get_nc():
    return _build_program()


def _fp8(a):
    return np.ascontiguousarray(
        np.clip(np.asarray(a, dtype=np.float32), -240.0, 240.0)
        .astype(ml_dtypes.float8_e4m3))


def kernel(x, c, w_ada, b_ada, w_qkv, w_proj, b_proj, w_mlp1, b_mlp1,
           w_mlp2, b_mlp2):
    nc = _get_nc()
    bf = ml_dtypes.bfloat16
    f32 = np.float32

    p = np.arange(128)
    w_qkv8 = _fp8((np.asarray(w_qkv, f32) * WS)
                  .reshape(4, 2, 128, 3 * H).transpose(2, 0, 1, 3))

    # --- w_proj row permutation matching the y8 layout ---
    phi = np.empty((128, 4, 2), np.int64)
    for ti in range(4):
        for j in range(2):
            phi[:, ti, j] = (4 * ti + 2 * j + p // 64) * CH + (p % 64)
    w_proj8 = _fp8((np.asarray(w_proj, f32) * WS)[phi])

    w1s = (np.asarray(w_mlp1, f32) * WS).reshape(4, 2, 128, 4 * H)\
        .transpose(2, 0, 1, 3)
    w_mlp18 = _fp8(w1s)
    w_mlp18l = _fp8(w1s - w_mlp18.astype(f32))
    w2s = (np.asarray(w_mlp2, f32) * WS).reshape(16, 2, 128, H)\
        .transpose(2, 0, 1, 3)
    w_mlp28 = _fp8(w2s)
    w_mlp28l = _fp8(w2s - w_mlp28.astype(f32))
    w_ada_t = np.ascontiguousarray(
        np.asarray(w_ada, f32).reshape(8, 128, 6, 1024)
        .transpose(1, 2, 0, 3).astype(bf))

    shared = {
        "w_ada_t": w_ada_t,
        "b_ada": np.ascontiguousarray(b_ada, dtype=f32),
        "w_qkv8": w_qkv8,
        "w_proj8": w_proj8,
        "b_proj": np.ascontiguousarray(b_proj, dtype=f32),
        "w_mlp18": w_mlp18,
        "w_mlp18l": w_mlp18l,
        "b_mlp1": np.ascontiguousarray(b_mlp1, dtype=f32),
        "w_mlp28": w_mlp28,
        "w_mlp28l": w_mlp28l,
        "b_mlp2": np.ascontiguousarray(b_mlp2, dtype=f32),
    }
    in_maps = []
    for bidx in range(N_CORES):
        m = dict(shared)
        m["xT8"] = np.ascontiguousarray(
            np.asarray(x[bidx], f32).T.reshape(8, 128, S)
            .transpose(1, 0, 2).astype(bf))
        m["cvec"] = np.ascontiguousarray(np.asarray(c[bidx], dtype=f32))
        in_maps.append(m)

    res = run_bass_kernel_spmd(
        nc, in_maps, core_ids=list(range(N_CORES)), trace=False
    )
    kernel.last_results = res

    out = np.empty((B, S, H), dtype=f32)
    for bidx in range(N_CORES):
        o = np.asarray(res.results[bidx]["outT8"])  # [128, 8, S]
        out[bidx] = o.transpose(1, 0, 2).reshape(H, S).T
    return out


if __name__ == "__main__":
    nc = _get_nc()
    print("program built ok")



# revision 52
# speedup vs baseline: 1.0933x; 1.0046x over previous
"""DiT block kernel for Trainium2, data-parallel over batch (8 cores, B=8).

v2: fp8 DoubleRow matmuls for qkv/scores/AV/proj/mlp1/mlp2 (2 K-chunks per
pass), LN statistics and adaLN GEMV computed with x/w_ada as the *stationary*
operand and a tiny moving operand (cost ~ output free size), per-token
quantities (LN rstd, softmax 1/den) produced directly in token-on-partition
layout via PE transposes of single rows, elementwise work spread across
DVE / ACT / GPSIMD.

Layouts (host-retiled):
  xT8      [128, 8, 1024]  bf16  x[b].T tiled: [p, kc, s] = x[b, s, kc*128+p]
  w_qkv8   [128, 4, 2, 3072] fp8 (x32), k/q column blocks permuted so that
           head h lives on partitions 32*(h%4).. with c split across the
           DoubleRow slot dim (c%32 on partitions, c//32 on slot)
  w_proj8  [128, 4, 2, 1024] fp8 (x32), rows permuted to match the y8 layout
           head h -> tile h//4, slot (h%4)//2, partition base 64*(h%2)
  w_mlp18  [128, 4, 2, 4096] fp8 (x32)
  w_mlp28  [128, 16, 2, 1024] fp8 (x32)
  w_ada_t  [128, 6, 8, 1024] bf16  [p, blk, kc, m] = w_ada[kc*128+p, blk*1024+m]
  outT8    [128, 8, 1024]  f32   [p, mc, s] = out[b, s, mc*128+p]

Scale bookkeeping: weights x32 in fp8. scoresT psum = 1024*k.q -> exp scale
1/(64*1024). AV y8 = 16*y_true (rd = 0.5/den). proj psum = 512*attn -> gate/512.
mlp1 psum = 32*pre -> gelu scale 1/32. mlp2 psum = 32*mlp -> gate/32.
"""

import os
import sys
import functools
from contextlib import ExitStack

import numpy as np

for _p in ("/opt/trn_rl_repo", "/root/.axon_site/_ro/trn_rl_repo"):
    if os.path.isdir(_p) and _p not in sys.path:
        sys.path.insert(0, _p)

import ml_dtypes  # noqa: E402
import concourse.bass as bass  # noqa: E402
from concourse import bacc  # noqa: E402
import concourse.tile as tile  # noqa: E402
from concourse import mybir  # noqa: E402
from concourse.bass_utils import run_bass_kernel_spmd  # noqa: E402

F32 = mybir.dt.float32
BF16 = mybir.dt.bfloat16
FP8 = mybir.dt.float8e4
AF = mybir.ActivationFunctionType
OP = mybir.AluOpType
DR = mybir.MatmulPerfMode.DoubleRow

B, S, H, NH, CH = 8, 1024, 1024, 16, 64
P = 128
KH = H // P          # 8 chunks over H
KS = S // P          # 8 chunks over S
EPS = 1e-6
N_CORES = 8
WS = 32.0            # fp8 weight scale

# scr2 (bf16) scratch layout: LN r/mr (2 LNs x 2 rows x S) then per-head rd
SCR2_LN = 0
SCR2_HEAD = 4 * S
SCR2_N = SCR2_HEAD + NH * S


def _build_program():
    nc = bacc.Bacc("TRN2", target_bir_lowering=False, debug=False)

    t = {}
    t["xT8"] = nc.dram_tensor("xT8", (P, KH, S), BF16, kind="ExternalInput").ap()
    t["cvec"] = nc.dram_tensor("cvec", (H,), F32, kind="ExternalInput").ap()
    t["w_ada_t"] = nc.dram_tensor("w_ada_t", (P, 6, KH, 1024), BF16,
                                  kind="ExternalInput").ap()
    t["b_ada"] = nc.dram_tensor("b_ada", (6 * H,), F32, kind="ExternalInput").ap()
    t["w_qkv8"] = nc.dram_tensor("w_qkv8", (P, 4, 2, 3 * H), FP8,
                                 kind="ExternalInput").ap()
    t["w_proj8"] = nc.dram_tensor("w_proj8", (P, 4, 2, H), FP8,
                                  kind="ExternalInput").ap()
    t["b_proj"] = nc.dram_tensor("b_proj", (H,), F32, kind="ExternalInput").ap()
    t["w_mlp18"] = nc.dram_tensor("w_mlp18", (P, 4, 2, 4 * H), FP8,
                                  kind="ExternalInput").ap()
    t["w_mlp18l"] = nc.dram_tensor("w_mlp18l", (P, 4, 2, 4 * H), FP8,
                                   kind="ExternalInput").ap()
    t["b_mlp1"] = nc.dram_tensor("b_mlp1", (4 * H,), F32, kind="ExternalInput").ap()
    t["w_mlp28"] = nc.dram_tensor("w_mlp28", (P, 16, 2, H), FP8,
                                  kind="ExternalInput").ap()
    t["w_mlp28l"] = nc.dram_tensor("w_mlp28l", (P, 16, 2, H), FP8,
                                   kind="ExternalInput").ap()
    t["b_mlp2"] = nc.dram_tensor("b_mlp2", (H,), F32, kind="ExternalInput").ap()
    t["outT8"] = nc.dram_tensor("outT8", (P, KH, S), F32, kind="ExternalOutput").ap()
    t["scr2"] = nc.dram_tensor("scr2", (SCR2_N,), BF16, kind="ExternalOutput").ap()

    with tile.TileContext(nc) as tc:
        _emit(tc, t)
    nc.compile()
    return nc


def _emit(tc, t):
    nc = tc.nc
    scr2 = t["scr2"]

    def pbcast(ap_1p, nparts):
        """Partition-broadcast view of a 1-partition (DRAM) AP."""
        return bass.AP(
            tensor=ap_1p.tensor, offset=ap_1p.offset,
            ap=[[0, nparts]] + list(ap_1p.ap[1:]),
        )

    def scr2_row(off, n):
        return scr2[off:off + n].rearrange("(a n) -> a n", a=1)

    def scr2_tok(off, n):
        """[128, n//128] view; flat[k*128+p] = element [p, k]."""
        return scr2[off:off + n].rearrange("(k p) -> p k", p=P)

    with ExitStack() as ctx:
        const = ctx.enter_context(tc.tile_pool(name="const", bufs=1))
        rows = ctx.enter_context(tc.tile_pool(name="rows", bufs=1))
        work = ctx.enter_context(tc.tile_pool(name="work", bufs=2))
        xpool = ctx.enter_context(tc.tile_pool(name="xpool", bufs=1))
        bcast = ctx.enter_context(tc.tile_pool(name="bcast", bufs=1))
        wmlp1_pool = ctx.enter_context(tc.tile_pool(name="wmlp1", bufs=1))
        wmlp1 = wmlp1_pool.tile([P, 4, 2, 4 * H], FP8, tag="wmlp1")
        wmlp1l = wmlp1_pool.tile([P, 4, 2, 4 * H], FP8, tag="wmlp1l")
        wada_ctx = ExitStack()
        wada_pool = wada_ctx.enter_context(tc.tile_pool(name="wada", bufs=1))

        # ---------------- constants ----------------------------------------
        ones_mv = const.tile([P, 1], BF16, tag="ones_mv")
        nc.vector.memset(ones_mv, 1.0)

        c_sb = const.tile([P, KH], F32, tag="c_sb")
        nc.gpsimd.dma_start(c_sb, t["cvec"].rearrange("(k p) -> p k", p=P))
        b_ada_sb = const.tile([P, 48], F32, tag="b_ada_sb")
        nc.gpsimd.dma_start(b_ada_sb, t["b_ada"].rearrange("(k p) -> p k", p=P))
        b_proj_sb = const.tile([P, KH], F32, tag="b_proj_sb")
        nc.gpsimd.dma_start(b_proj_sb, t["b_proj"].rearrange("(k p) -> p k", p=P))
        b_mlp1_sb = const.tile([P, 32], F32, tag="b_mlp1_sb")
        nc.gpsimd.dma_start(b_mlp1_sb, t["b_mlp1"].rearrange("(k p) -> p k", p=P))
        b_mlp2_sb = const.tile([P, KH], F32, tag="b_mlp2_sb")
        nc.gpsimd.dma_start(b_mlp2_sb, t["b_mlp2"].rearrange("(k p) -> p k", p=P))

        # ---------------- input x + first w_ada half-blocks ----------------
        xall = xpool.tile([P, KH, S], BF16, tag="xall")
        nc.sync.dma_start(xall[:, 0:4, :], t["xT8"][:, 0:4, :])
        nc.sync.dma_start(xall[:, 4:8, :], t["xT8"][:, 4:8, :])

        # w_ada streamed as 12 half-blocks [P, KH, 512] (4 cmod cols each)
        def wada_dma(hb, eng=None):
            wt = wada_pool.tile([P, KH, 512], BF16, tag="wada")
            blk, mlo = hb // 2, (hb % 2) * 512
            (eng or nc.gpsimd).dma_start(
                wt, t["w_ada_t"][:, blk, :, mlo:mlo + 512])
            return wt

        wada_tiles = {hb: wada_dma(hb, nc.sync) for hb in range(2)}

        # ---------------- silu(c) ------------------------------------------
        sc_sb = const.tile([P, KH], BF16, tag="sc_sb")
        nc.scalar.activation(sc_sb, c_sb, AF.Silu)

        # attention SBUF tiles (allocated early for pool stack order)
        att_ctx = ExitStack()
        kq_pool = att_ctx.enter_context(tc.tile_pool(name="kq", bufs=1))
        k8 = [kq_pool.tile([P, 2, S], FP8, tag=f"k8_{i}", name=f"k8_{i}")
              for i in range(KS)]
        q8 = [kq_pool.tile([P, 2, S], FP8, tag=f"q8_{i}", name=f"q8_{i}")
              for i in range(KS)]
        for i in range(KS):
            nc.vector.memset(k8[i][:, 1, :].bitcast(mybir.dt.uint32), 0)
            nc.vector.memset(q8[i][:, 1, :].bitcast(mybir.dt.uint32), 0)
        v2 = [kq_pool.tile([P, 2, NH // 2, 2, CH + 32], FP8, tag=f"v2_{i}",
                           name=f"v2_{i}")
              for i in range(4)]
        for i in range(4):
            nc.vector.memset(v2[i][:, :, :, :, CH:CH + 32], 2.0)
        y8 = [kq_pool.tile([P, 2, S], FP8, tag=f"y8_{i}", name=f"y8_{i}")
              for i in range(4)]

        # ---------------- phase-1 psum pools --------------------------------
        ph1 = ExitStack()
        ps_ada = ph1.enter_context(tc.tile_pool(name="ps_ada", bufs=1, space="PSUM"))
        psada = ps_ada.tile([P, 48], F32, tag="ada")
        cmod = const.tile([P, 48], F32, tag="cmod")

        ln1_ps = ExitStack()
        ps_ln = ln1_ps.enter_context(tc.tile_pool(name="ps_ln", bufs=1, space="PSUM"))

        # ---------------- LN statistics (x stationary, ones moving) --------
        def ln_stats(pool, src, xsq_tag):
            """Returns psum [128, KS*KH] partial sums & sumsq (col tcv*KH+kc),
            token s = tc*128 + p. Each matmul is an independent start/stop
            group: interleaved accumulation in one PSUM bank is NOT safe (the
            start flag marks the whole 2 KiB bank pending-zero, wiping other
            columns' later accumulating writes), but completed columns' data
            survives subsequent starts."""
            pss = pool.tile([P, KS * KH], F32, tag="ln_s")
            psq = pool.tile([P, KS * KH], F32, tag="ln_q")
            for kc in range(KH):
                xsq = work.tile([P, S], BF16, tag=xsq_tag)
                nc.scalar.activation(xsq, src[:, kc, :], AF.Square)
                for tcv in range(KS):
                    sl = slice(tcv * P, (tcv + 1) * P)
                    col = tcv * KH + kc
                    nc.tensor.matmul(
                        pss[:, col:col + 1], lhsT=src[:, kc, sl], rhs=ones_mv,
                        start=True, stop=True,
                    )
                    nc.tensor.matmul(
                        psq[:, col:col + 1], lhsT=xsq[:, sl], rhs=ones_mv,
                        start=True, stop=True,
                    )
            return pss, psq

        def ln_finish(pss, psq, o2_base):
            """rstd & mean*rstd from [128, KS] stats; bf16 via scr2 to
            partition-broadcast tiles [128, S]."""
            pssum = rows.tile([P, KS], F32, tag="pssum")
            nc.vector.tensor_reduce(
                pssum, pss.rearrange("p (t k) -> p t k", t=KS),
                axis=mybir.AxisListType.X, op=OP.add)
            psqs = rows.tile([P, KS], F32, tag="psqs")
            nc.vector.tensor_reduce(
                psqs, psq.rearrange("p (t k) -> p t k", t=KS),
                axis=mybir.AxisListType.X, op=OP.add)
            m = rows.tile([P, KS], F32, tag="m_tok")
            nc.vector.tensor_scalar(out=m, in0=pssum, scalar1=1.0 / H,
                                    scalar2=0.0, op0=OP.mult, op1=OP.bypass)
            v = rows.tile([P, KS], F32, tag="v_tok")
            nc.vector.tensor_scalar(out=v, in0=psqs, scalar1=1.0 / H,
                                    scalar2=EPS, op0=OP.mult, op1=OP.add)
            msq = rows.tile([P, KS], F32, tag="msq_tok")
            nc.vector.tensor_tensor(msq, m, m, OP.mult)
            nc.vector.tensor_tensor(v, v, msq, OP.subtract)
            r = rows.tile([P, KS], F32, tag="r_tok")
            nc.vector.tensor_scalar(out=r, in0=v, scalar1=-0.5, scalar2=1.5,
                                    op0=OP.mult, op1=OP.add)
            s = rows.tile([P, KS], F32, tag="s_tok")
            for _ in range(2):
                nc.vector.tensor_tensor(s, r, r, OP.mult)
                nc.vector.tensor_tensor(s, s, v, OP.mult)
                nc.vector.tensor_scalar(out=s, in0=s, scalar1=-0.5, scalar2=1.5,
                                        op0=OP.mult, op1=OP.add)
                nc.vector.tensor_tensor(r, r, s, OP.mult)
            nc.vector.tensor_tensor(m, m, r, OP.mult)  # m <- m * r
            rb16 = rows.tile([P, KS], BF16, tag="rb16")
            nc.vector.tensor_copy(rb16, r)
            mb16 = rows.tile([P, KS], BF16, tag="mb16")
            nc.vector.tensor_copy(mb16, m)
            nc.gpsimd.dma_start(scr2_tok(o2_base, S), rb16)
            nc.gpsimd.dma_start(scr2_tok(o2_base + S, S), mb16)
            rrow = rows.tile([1, S], BF16, tag="rrow")
            mrow = rows.tile([1, S], BF16, tag="mrow")
            nc.sync.dma_start(rrow, scr2_row(o2_base, S))
            nc.sync.dma_start(mrow, scr2_row(o2_base + S, S))
            r_b = bcast.tile([P, S], BF16, tag="r_b")
            mr_b = bcast.tile([P, S], BF16, tag="mr_b")
            nc.gpsimd.partition_broadcast(r_b, rrow)
            nc.gpsimd.partition_broadcast(mr_b, mrow)
            return r_b, mr_b

        pss1, psq1 = ln_stats(ps_ln, xall, "xsq")

        # adaLN GEMV: one half-block = 4 cmod columns of 128
        def ada_halfblock(hb, wt, ps, base):
            for mcol in range(4):
                col = hb * 4 + mcol - base
                for kc in range(KH):
                    nc.tensor.matmul(
                        ps[:, col:col + 1],
                        lhsT=wt[:, kc, mcol * P:(mcol + 1) * P],
                        rhs=sc_sb[:, kc:kc + 1],
                        start=(kc == 0), stop=(kc == KH - 1),
                    )
            nc.vector.tensor_tensor(
                cmod[:, hb * 4:(hb + 1) * 4],
                ps[:, hb * 4 - base:(hb + 1) * 4 - base],
                b_ada_sb[:, hb * 4:(hb + 1) * 4], OP.add,
            )

        for hb in range(2):  # shift_msa
            ada_halfblock(hb, wada_tiles.pop(hb), psada, 0)

        r1_b, mr1_b = ln_finish(pss1, psq1, SCR2_LN)
        for hb in range(2, 4):  # scale_msa, behind the r/mr loads in the FIFO
            ada_halfblock(hb, wada_dma(hb), psada, 0)
        ln1_ps.close()

        sc1 = const.tile([P, 16], F32, tag="sc1")  # 1+scale_msa | 1+scale_mlp
        nc.scalar.add(sc1[:, 0:8], cmod[:, 8:16], 1.0)

        # ---------------- z1 modulate + qkv (DoubleRow fp8) ----------------
        zpool = ExitStack()
        z1_pool = zpool.enter_context(tc.tile_pool(name="z1", bufs=1))
        z1 = [z1_pool.tile([P, 2, S], FP8, tag=f"z1_{i}", name=f"z1_{i}")
              for i in range(4)]

        wqkv_ctx = ExitStack()
        wqkv_pool = wqkv_ctx.enter_context(tc.tile_pool(name="wqkv", bufs=1))
        wqkv = wqkv_pool.tile([P, 4, 2, 3 * H], FP8, tag="wqkv")
        for sec in range(3):
            nc.gpsimd.dma_start(
                wqkv[:, :, :, sec * H:(sec + 1) * H],
                t["w_qkv8"][:, :, :, sec * H:(sec + 1) * H],
            )

        def modulate(dst, src, r_b, mr_b, col, shift_ap, sl):
            tm = work.tile([P, S], BF16, tag="mod_tm")
            nc.vector.tensor_tensor(tm[:, sl], src[:, sl], r_b[:, sl], OP.mult)
            nc.vector.tensor_tensor(tm[:, sl], tm[:, sl], mr_b[:, sl],
                                    OP.subtract)
            nc.scalar.activation(dst[:, sl], tm[:, sl], AF.Identity,
                                 bias=shift_ap, scale=sc1[:, col:col + 1])

        for half in range(2):
            sl = slice(half * 512, (half + 1) * 512)
            for kc in range(KH):
                modulate(z1[kc // 2][:, kc % 2, :], xall[:, kc, :], r1_b, mr1_b,
                         kc, cmod[:, kc:kc + 1], sl)

        ps_mm_ctx = ExitStack()
        ps_mm = ps_mm_ctx.enter_context(
            tc.tile_pool(name="ps_mm", bufs=3, space="PSUM"))

        for oc in range(16):  # 8 k-chunks then 8 q-chunks
            ps = ps_mm.tile([P, S], F32, tag="mm")
            for half in range(2):
                sl = slice(half * 512, (half + 1) * 512)
                for kcp in range(4):
                    nc.tensor.matmul(
                        ps[:, sl],
                        lhsT=wqkv[:, kcp, :, oc * P:(oc + 1) * P],
                        rhs=z1[kcp][:, :, sl],
                        start=(kcp == 0), stop=(kcp == 3), perf_mode=DR,
                    )
            dst = k8 if oc < 8 else q8
            nc.scalar.copy(dst[oc % 8][:, 0, :], ps)

        for sc in range(KS):  # v, token-major
            ps = ps_mm.tile([P, S], F32, tag="mm")
            for half in range(2):
                sl = slice(2048 + half * 512, 2048 + (half + 1) * 512)
                osl = slice(half * 512, (half + 1) * 512)
                for kcp in range(4):
                    nc.tensor.matmul(
                        ps[:, osl],
                        lhsT=z1[kcp][:, :, sc * P:(sc + 1) * P],
                        rhs=wqkv[:, kcp, :, sl],
                        start=(kcp == 0), stop=(kcp == 3), perf_mode=DR,
                    )
            nc.scalar.copy(
                v2[sc // 2][:, sc % 2, :, :, 0:CH],
                ps.rearrange("p (hp two c) -> p hp two c", hp=NH // 2, two=2),
            )
        ps_mm_ctx.close()
        wqkv_ctx.close()
        zpool.close()
        ph1.close()

        # ---------------- attention ----------------------------------------
        wexp_pool = att_ctx.enter_context(tc.tile_pool(name="wexp", bufs=8))
        att_tmp = att_ctx.enter_context(tc.tile_pool(name="att_tmp", bufs=3))
        rdb_pool = att_ctx.enter_context(tc.tile_pool(name="rdb", bufs=3))
        wproj_pool = att_ctx.enter_context(tc.tile_pool(name="wproj", bufs=1))
        wproj = wproj_pool.tile([P, 4, 2, H], FP8, tag="wproj")

        att_ps = ExitStack()
        spool = att_ps.enter_context(tc.tile_pool(name="spool", bufs=3, space="PSUM"))
        avpool = att_ps.enter_context(tc.tile_pool(name="avpool", bufs=1, space="PSUM"))

        DVE_EXP_KC = ()

        def head_scores(h):
            ti, off = h // 2, 64 * (h % 2)
            prow = slice(off, off + CH)
            wexp = []
            for kcp in range(4):
                wt = wexp_pool.tile([P, 2, S], FP8, tag="wexp")
                for j in range(2):
                    kc = 2 * kcp + j
                    ps_s = spool.tile([P, S], F32, tag="ps")
                    for half in range(2):
                        sl = slice(half * 512, (half + 1) * 512)
                        nc.tensor.matmul(
                            ps_s[:, sl],
                            lhsT=k8[ti][prow, :, kc * P:(kc + 1) * P],
                            rhs=q8[ti][prow, :, sl],
                            start=True, stop=True, perf_mode=DR,
                        )
                    if kc in DVE_EXP_KC:
                        # exp(t) ~ 1 + t(1 + t/2), |t| < 0.5 (err < 1e-3)
                        tq = work.tile([P, S], BF16, tag="mod_tm")
                        nc.vector.tensor_scalar(
                            out=tq, in0=ps_s,
                            scalar1=1.0 / (64.0 * WS * WS), scalar2=0.0,
                            op0=OP.mult, op1=OP.bypass)
                        uq = work.tile([P, S], BF16, tag="res_tmp")
                        nc.vector.tensor_scalar(
                            out=uq, in0=tq, scalar1=0.5, scalar2=1.0,
                            op0=OP.mult, op1=OP.add)
                        nc.vector.tensor_tensor(uq, tq, uq, OP.mult)
                        nc.vector.tensor_scalar(
                            out=wt[:, j, :], in0=uq, scalar1=1.0, scalar2=1.0,
                            op0=OP.mult, op1=OP.add)
                    else:
                        nc.scalar.activation(wt[:, j, :], ps_s, AF.Exp,
                                             scale=1.0 / (64.0 * WS * WS))
                wexp.append(wt)
            return wexp

        def head_av(h, wexp):
            """AV matmul with the softmax denominator fused in: the
            stationary operand is [v_head | 32 ones-cols valued 2.0], so
            output rows 0:64 are y_unnorm and rows 64:96 are 2*den — one
            DoubleRow group at base partition 0 (ISA-safe), no extra cost
            (matmul cost is output free size only)."""
            ps_y = avpool.tile([P, S], F32, tag="ps_y")
            for half in range(2):
                sl = slice(half * 512, (half + 1) * 512)
                for kcp in range(4):
                    nc.tensor.matmul(
                        ps_y[0:CH + 32, sl],
                        lhsT=v2[kcp][:, :, h // 2, h % 2, :],
                        rhs=wexp[kcp][:, :, sl],
                        start=(kcp == 0), stop=(kcp == 3), perf_mode=DR,
                    )
            drow = att_tmp.tile([1, S], BF16, tag="drow", bufs=2)
            with nc.allow_low_precision(reason="softmax 1/den in bf16"):
                nc.vector.reciprocal(drow, ps_y[CH:CH + 1, :])  # 0.5/den
            rdb = rdb_pool.tile([P, S], BF16, tag="rdb")
            nc.gpsimd.partition_broadcast(rdb, drow)
            return ps_y, rdb

        def head_norm(h, ps_y, rdb):
            ti, j, off = h // 4, (h % 4) // 2, 64 * (h % 2)
            nc.vector.tensor_tensor(
                y8[ti][off:off + CH, j, :],
                ps_y[0:CH, :], rdb[0:CH, :], OP.mult,
            )

        def late_streams(step):
            if step == 2:
                nc.gpsimd.dma_start(wproj, t["w_proj8"])
            elif 4 <= step < 12:  # w_ada blocks 4..11
                wada_tiles[step] = wada_dma(step)
            elif 12 <= step < 20:  # w_mlp1 hi, 1MB pieces
                i = step - 12
                nc.gpsimd.dma_start(wmlp1[:, i // 2, :, (i % 2) * 2048:
                                          (i % 2) * 2048 + 2048],
                                    t["w_mlp18"][:, i // 2, :, (i % 2) * 2048:
                                                 (i % 2) * 2048 + 2048])
            elif 20 <= step < 28:  # w_mlp1 lo
                i = step - 20
                nc.gpsimd.dma_start(wmlp1l[:, i // 2, :, (i % 2) * 2048:
                                           (i % 2) * 2048 + 2048],
                                    t["w_mlp18l"][:, i // 2, :, (i % 2) * 2048:
                                                  (i % 2) * 2048 + 2048])

            if 6 <= step < 14:  # adaLN tail rides the scores psum ring
                hb = step - 2
                psx = spool.tile([P, S], F32, tag="ps")
                ada_halfblock(hb, wada_tiles.pop(hb), psx, hb * 4)

        st = {}
        for step in range(28):
            late_streams(step)
            if step < NH:
                st[step] = {"wexp": head_scores(step)}
            if 1 <= step and step - 1 < NH:
                hh = step - 1
                ps_y, rdb = head_av(hh, st[hh].pop("wexp"))
                st[hh]["ps_y"], st[hh]["rdb"] = ps_y, rdb
            if 2 <= step and step - 2 < NH:
                hh = step - 2
                head_norm(hh, st[hh].pop("ps_y"), st[hh].pop("rdb"))
                del st[hh]
        att_ps.close()


        nc.scalar.add(sc1[:, 8:16], cmod[:, 32:40], 1.0)
        gpr = const.tile([P, KH], F32, tag="gpr")
        nc.vector.tensor_scalar(out=gpr, in0=cmod[:, 16:24],
                                scalar1=1.0 / 512.0, scalar2=0.0,
                                op0=OP.mult, op1=OP.bypass)
        gpb = const.tile([P, KH], F32, tag="gpb")
        nc.vector.tensor_tensor(gpb, cmod[:, 16:24], b_proj_sb, OP.mult)
        gmr = const.tile([P, KH], F32, tag="gmr")
        nc.vector.tensor_scalar(out=gmr, in0=cmod[:, 40:48],
                                scalar1=1.0 / 32.0, scalar2=0.0,
                                op0=OP.mult, op1=OP.bypass)
        gmb = const.tile([P, KH], F32, tag="gmb")
        nc.vector.tensor_tensor(gmb, cmod[:, 40:48], b_mlp2_sb, OP.mult)

        # ---------------- proj + gated residual + LN2 stats -----------------
        ph3 = ExitStack()
        ps_pr = ph3.enter_context(tc.tile_pool(name="ps_pr", bufs=2, space="PSUM"))
        ps_ln2 = ph3.enter_context(tc.tile_pool(name="ps_ln2", bufs=1, space="PSUM"))
        pss2 = ps_ln2.tile([P, KS * KH], F32, tag="ln_s")
        psq2 = ps_ln2.tile([P, KS * KH], F32, tag="ln_q")
        for mc in range(KH):
            ps = ps_pr.tile([P, S], F32, tag="mm")
            for half in range(2):
                sl = slice(half * 512, (half + 1) * 512)
                for ti in range(4):
                    nc.tensor.matmul(
                        ps[:, sl],
                        lhsT=wproj[:, ti, :, mc * P:(mc + 1) * P],
                        rhs=y8[ti][:, :, sl],
                        start=(ti == 0), stop=(ti == 3), perf_mode=DR,
                    )
            tp = work.tile([P, S], BF16, tag="res_tmp")
            nc.scalar.activation(tp, ps, AF.Identity,
                                 bias=gpb[:, mc:mc + 1],
                                 scale=gpr[:, mc:mc + 1])
            nc.vector.tensor_tensor(xall[:, mc, :], xall[:, mc, :], tp, OP.add)
            # LN2 statistics for this chunk right away
            xsq = work.tile([P, S], BF16, tag="xsq")
            nc.vector.tensor_tensor(xsq, xall[:, mc, :], xall[:, mc, :],
                                    OP.mult)
            for tcv in range(KS):
                sl = slice(tcv * P, (tcv + 1) * P)
                col = tcv * KH + mc
                nc.tensor.matmul(
                    pss2[:, col:col + 1], lhsT=xall[:, mc, sl], rhs=ones_mv,
                    start=True, stop=True,
                )
                nc.tensor.matmul(
                    psq2[:, col:col + 1], lhsT=xsq[:, sl], rhs=ones_mv,
                    start=True, stop=True,
                )
        att_ctx.close()

        # ---------------- LN2 finish + modulate z2 + MLP --------------------
        ph4 = ExitStack()
        r2_b, mr2_b = ln_finish(pss2, psq2, SCR2_LN + 2 * S)
        ph3.close()
        wada_ctx.close()

        h_pool = ph4.enter_context(tc.tile_pool(name="h8", bufs=1))
        h8 = [h_pool.tile([P, 2, S], FP8, tag=f"h8_{i}", name=f"h8_{i}")
              for i in range(16)]
        z2_pool = ph4.enter_context(tc.tile_pool(name="z2", bufs=1))
        z2 = [z2_pool.tile([P, 2, S], FP8, tag=f"z2_{i}", name=f"z2_{i}")
              for i in range(4)]
        z2l = [z2_pool.tile([P, 2, S], FP8, tag=f"z2l_{i}", name=f"z2l_{i}")
               for i in range(4)]
        for half in range(2):
            sl = slice(half * 512, (half + 1) * 512)
            for kc in range(KH):
                # z_bf (bf16) -> z_hi (fp8) -> z_lo = fp8(z_bf - z_hi)
                tm = work.tile([P, S], BF16, tag="mod_tm")
                nc.vector.tensor_tensor(tm[:, sl], xall[:, kc, sl],
                                        r2_b[:, sl], OP.mult)
                nc.vector.tensor_tensor(tm[:, sl], tm[:, sl], mr2_b[:, sl],
                                        OP.subtract)
                zbf = work.tile([P, S], BF16, tag="stage_bf")
                nc.vector.tensor_scalar(
                    out=zbf[:, sl], in0=tm[:, sl],
                    scalar1=sc1[:, 8 + kc:8 + kc + 1],
                    scalar2=cmod[:, 24 + kc:24 + kc + 1],
                    op0=OP.mult, op1=OP.add,
                )
                zhi = z2[kc // 2][:, kc % 2, :]
                nc.scalar.copy(zhi[:, sl], zbf[:, sl])
                nc.gpsimd.tensor_tensor(z2l[kc // 2][:, kc % 2, sl],
                                        zbf[:, sl], zhi[:, sl], OP.subtract)

        wmlp2_pool = ph4.enter_context(tc.tile_pool(name="wmlp2", bufs=2))
        otmp_pool = ph4.enter_context(tc.tile_pool(name="otmp", bufs=2))

        def w2_blk_dma(mc):  # 1 out-chunk of hi+lo
            bh = wmlp2_pool.tile([P, 16, 2, P], FP8, tag="w2hi")
            nc.sync.dma_start(bh, t["w_mlp28"][:, :, :, mc * P:(mc + 1) * P])
            bl = wmlp2_pool.tile([P, 16, 2, P], FP8, tag="w2lo")
            nc.sync.dma_start(bl, t["w_mlp28l"][:, :, :, mc * P:(mc + 1) * P])
            return bh, bl

        w2blk = {0: w2_blk_dma(0), 1: w2_blk_dma(1)}

        m1_ctx = ExitStack()
        ps_m1 = m1_ctx.enter_context(tc.tile_pool(name="ps_m1", bufs=3, space="PSUM"))

        for mc in range(32):
            ps = ps_m1.tile([P, S], F32, tag="mm")
            for half in range(2):
                sl = slice(half * 512, (half + 1) * 512)
                for p_ in range(3):
                    wsrc = wmlp1 if p_ != 1 else wmlp1l
                    zsrc = z2 if p_ != 2 else z2l
                    for kcp in range(4):
                        nc.tensor.matmul(
                            ps[:, sl],
                            lhsT=wsrc[:, kcp, :, mc * P:(mc + 1) * P],
                            rhs=zsrc[kcp][:, :, sl],
                            start=(p_ == 0 and kcp == 0),
                            stop=(p_ == 2 and kcp == 3), perf_mode=DR,
                        )
            nc.scalar.activation(
                h8[mc // 2][:, mc % 2, :], ps, AF.Gelu_apprx_tanh,
                bias=b_mlp1_sb[:, mc:mc + 1], scale=1.0 / WS,
            )
        m1_ctx.close()

        w2_ctx = ExitStack()
        ps_m2 = w2_ctx.enter_context(tc.tile_pool(name="ps_m2", bufs=3, space="PSUM"))
        for mc in range(KH):
            if mc + 2 < KH:
                w2blk[mc + 2] = w2_blk_dma(mc + 2)
            bh, bl = w2blk.pop(mc)
            ps = ps_m2.tile([P, S], F32, tag="mm")
            off = 0
            for half in range(2):
                sl = slice(half * 512, (half + 1) * 512)
                for p_ in range(2):
                    wsrc = bh if p_ != 1 else bl
                    for kcp in range(16):
                        nc.tensor.matmul(
                            ps[:, sl],
                            lhsT=wsrc[:, kcp, :, off:off + P],
                            rhs=h8[kcp][:, :, sl],
                            start=(p_ == 0 and kcp == 0),
                            stop=(p_ == 1 and kcp == 15), perf_mode=DR,
                        )
            tp = work.tile([P, S], BF16, tag="res_tmp")
            nc.scalar.activation(tp, ps, AF.Identity,
                                 bias=gmb[:, mc:mc + 1],
                                 scale=gmr[:, mc:mc + 1])
            ot = otmp_pool.tile([P, S], F32, tag="ot")
            nc.vector.tensor_tensor(ot, xall[:, mc, :], tp, OP.add)
            nc.sync.dma_start(t["outT8"][:, mc, :], ot)
        w2_ctx.close()
        ph4.close()


@functools.lru_cache(maxsize=1)
def _get_nc():
    return _build_program()


def _fp8(a):
    return np.ascontiguousarray(
        np.clip(np.asarray(a, dtype=np.float32), -240.0, 240.0)
        .astype(ml_dtypes.float8_e4m3))


def kernel(x, c, w_ada, b_ada, w_qkv, w_proj, b_proj, w_mlp1, b_mlp1,
           w_mlp2, b_mlp2):
    nc = _get_nc()
    bf = ml_dtypes.bfloat16
    f32 = np.float32

    p = np.arange(128)
    w_qkv8 = _fp8((np.asarray(w_qkv, f32) * WS)
                  .reshape(4, 2, 128, 3 * H).transpose(2, 0, 1, 3))

    # --- w_proj row permutation matching the y8 layout ---
    phi = np.empty((128, 4, 2), np.int64)
    for ti in range(4):
        for j in range(2):
            phi[:, ti, j] = (4 * ti + 2 * j + p // 64) * CH + (p % 64)
    w_proj8 = _fp8((np.asarray(w_proj, f32) * WS)[phi])

    w1s = (np.asarray(w_mlp1, f32) * WS).reshape(4, 2, 128, 4 * H)\
        .transpose(2, 0, 1, 3)
    w_mlp18 = _fp8(w1s)
    w_mlp18l = _fp8(w1s - w_mlp18.astype(f32))
    w2s = (np.asarray(w_mlp2, f32) * WS).reshape(16, 2, 128, H)\
        .transpose(2, 0, 1, 3)
    w_mlp28 = _fp8(w2s)
    w_mlp28l = _fp8(w2s - w_mlp28.astype(f32))
    w_ada_t = np.ascontiguousarray(
        np.asarray(w_ada, f32).reshape(8, 128, 6, 1024)
        .transpose(1, 2, 0, 3).astype(bf))

    shared = {
        "w_ada_t": w_ada_t,
        "b_ada": np.ascontiguousarray(b_ada, dtype=f32),
        "w_qkv8": w_qkv8,
        "w_proj8": w_proj8,
        "b_proj": np.ascontiguousarray(b_proj, dtype=f32),
        "w_mlp18": w_mlp18,
        "w_mlp18l": w_mlp18l,
        "b_mlp1": np.ascontiguousarray(b_mlp1, dtype=f32),
        "w_mlp28": w_mlp28,
        "w_mlp28l": w_mlp28l,
        "b_mlp2": np.ascontiguousarray(b_mlp2, dtype=f32),
    }
    in_maps = []
    for bidx in range(N_CORES):
        m = dict(shared)
        m["xT8"] = np.ascontiguousarray(
            np.asarray(x[bidx], f32).T.reshape(8, 128, S)
            .transpose(1, 0, 2).astype(bf))
        m["cvec"] = np.ascontiguousarray(np.asarray(c[bidx], dtype=f32))
        in_maps.append(m)

    res = run_bass_kernel_spmd(
        nc, in_maps, core_ids=list(range(N_CORES)), trace=False
    )
    kernel.last_results = res

    out = np.empty((B, S, H), dtype=f32)
    for bidx in range(N_CORES):
        o = np.asarray(res.results[bidx]["outT8"])  # [128, 8, S]
        out[bidx] = o.transpose(1, 0, 2).reshape(H, S).T
    return out


if __name__ == "__main__":
    nc = _get_nc()
    print("program built ok")

